# revision 1
# baseline (speedup 1.0000x reference)
"""Llama4-style MoE (T=1024, H=2048, I=4096, E=8, top-1) on 8 trn2 NeuronCores.

Sharding: expert-parallel. Core e owns expert e's weights (96 MB) plus a
1/8 I-shard of the shared expert (12 MB). Host computes top-1 routing
(tiny [1024,8] matmul) and dispatches each expert's tokens (scaled by the
sigmoid router score, padded to capacity C) to its core. Each core returns
its expert's MLP output plus a partial shared-expert output; host sums the
partials and scatters the routed rows back.

Device kernel works entirely in "transposed activation" space
([feature, token], feature on partitions) so no on-chip transposes are
needed; matmuls use fp32r (FP22 multiply, fp32 accumulate) with moving
free dim >= 256 for full PE speed.
"""

import numpy as np

T, H, I, E = 1024, 2048, 4096, 8
P = 128
ISH = I // E          # 512  shared-expert I-shard per core
KH = H // P           # 16
MI = I // P           # 32
MH = H // P           # 16
KSH = ISH // P        # 4

_BASS_CACHE = {}
LAST_RESULT = None    # BassKernelResults of the most recent run (for test harness)


def _build_bass(C, repeats=1, wbufs=16, xbufs=1):
    import concourse.bass as bass
    import concourse.mybir as mybir
    import concourse.tile as tile

    f32 = mybir.dt.float32
    f32r = mybir.dt.float32r
    SILU = mybir.ActivationFunctionType.Silu
    MULT = mybir.AluOpType.mult

    nc = bass.Bass(trn_type="TRN2", name=f"moe_ep_c{C}_r{repeats}")

    xe_t = nc.dram_tensor("xe_t", [H, C], f32r, kind="ExternalInput")
    wg = nc.dram_tensor("wg", [H, I], f32r, kind="ExternalInput")
    wu = nc.dram_tensor("wu", [H, I], f32r, kind="ExternalInput")
    wd = nc.dram_tensor("wd", [I, H], f32r, kind="ExternalInput")
    x_t = nc.dram_tensor("x_t", [H, T], f32r, kind="ExternalInput")
    wsg = nc.dram_tensor("wsg", [H, ISH], f32r, kind="ExternalInput")
    wsu = nc.dram_tensor("wsu", [H, ISH], f32r, kind="ExternalInput")
    wsd = nc.dram_tensor("wsd", [ISH, H], f32r, kind="ExternalInput")
    ro_t = nc.dram_tensor("ro_t", [H, C], f32, kind="ExternalOutput")
    sp_t = nc.dram_tensor("sp_t", [H, T], f32, kind="ExternalOutput")

    with tile.TileContext(nc) as tc:
        from contextlib import ExitStack

        with ExitStack() as ctx:
            const = ctx.enter_context(tc.tile_pool(name="const", bufs=1))
            wpool = ctx.enter_context(tc.tile_pool(name="wpool", bufs=wbufs))
            xtp = ctx.enter_context(tc.tile_pool(name="xtp", bufs=xbufs))
            hsp = ctx.enter_context(tc.tile_pool(name="hsp", bufs=2))
            hbuf = ctx.enter_context(tc.tile_pool(name="hbuf", bufs=2))
            outp = ctx.enter_context(tc.tile_pool(name="outp", bufs=4))
            psum = ctx.enter_context(tc.tile_pool(name="psum", bufs=8, space="PSUM"))

            # --- resident tensors ---
            xeT = const.tile([P, KH, C], f32r)  # routed tokens, transposed
            xe_view = xe_t.ap().rearrange("(k p) c -> p k c", p=P)
            for k in range(KH):
                nc.sync.dma_start(out=xeT[:, k, :], in_=xe_view[:, k, :])

            wsd_sb = const.tile([P, KSH, H], f32r)  # shared down-proj (4 MB)
            wsd_view = wsd.ap().rearrange("(k p) h -> p k h", p=P)
            for k in range(KSH):
                nc.sync.dma_start(out=wsd_sb[:, k, :], in_=wsd_view[:, k, :])

            hTp = ctx.enter_context(tc.tile_pool(name="hTp", bufs=1))

            ro_view = ro_t.ap().rearrange("(m p) c -> p m c", p=P)
            sp_view = sp_t.ap().rearrange("(m p) t -> p m t", p=P)

            for rep in range(repeats):
              hT = hTp.tile([P, MI, C], f32r, tag="hT", name=f"hT{rep}")  # routed hidden
              # ---------- routed expert: gate/up -> hT ----------
            for g in range(I // 512):  # 8 groups of 4 I-tiles
                gps = [psum.tile([P, 2, C], f32, tag="ps", name=f"gps{g}_{h}") for h in range(2)]
                ups = [psum.tile([P, 2, C], f32, tag="ps", name=f"ups{g}_{h}") for h in range(2)]
                for k in range(KH):
                    wg_b = wpool.tile([P, 512], f32r, tag="wblk", name=f"wgb{g}_{k}")
                    nc.sync.dma_start(out=wg_b, in_=wg.ap()[k * P:(k + 1) * P, g * 512:(g + 1) * 512])
                    wu_b = wpool.tile([P, 512], f32r, tag="wblk", name=f"wub{g}_{k}")
                    nc.sync.dma_start(out=wu_b, in_=wu.ap()[k * P:(k + 1) * P, g * 512:(g + 1) * 512])
                    for mi in range(4):
                        st = dict(start=(k == 0 and mi % 2 == 0), stop=(k == KH - 1))
                        nc.tensor.matmul(gps[mi // 2][:, mi % 2, :], wg_b[:, mi * P:(mi + 1) * P], xeT[:, k, :], **st)
                    for mi in range(4):
                        st = dict(start=(k == 0 and mi % 2 == 0), stop=(k == KH - 1))
                        nc.tensor.matmul(ups[mi // 2][:, mi % 2, :], wu_b[:, mi * P:(mi + 1) * P], xeT[:, k, :], **st)
                for h in range(2):
                    h_sb = hbuf.tile([P, 2, C], f32, tag="hsb", name=f"hsb{g}_{h}")
                    nc.scalar.activation(out=h_sb, in_=gps[h], func=SILU)
                    nc.vector.tensor_tensor(hT[:, g * 4 + 2 * h: g * 4 + 2 * h + 2, :], h_sb, ups[h], MULT)

            # ---------- routed expert: down -> ro_t ----------
            for g2 in range(4):  # H groups of 512
                dps = [psum.tile([P, 2, C], f32, tag="ps", name=f"dps{g2}_{h}") for h in range(2)]
                for k2 in range(MI):  # 32
                    wd_b = wpool.tile([P, 512], f32r, tag="wblk", name=f"wdb{g2}_{k2}")
                    nc.sync.dma_start(out=wd_b, in_=wd.ap()[k2 * P:(k2 + 1) * P, g2 * 512:(g2 + 1) * 512])
                    for mi in range(4):
                        st = dict(start=(k2 == 0 and mi % 2 == 0), stop=(k2 == MI - 1))
                        nc.tensor.matmul(dps[mi // 2][:, mi % 2, :], wd_b[:, mi * P:(mi + 1) * P], hT[:, k2, :], **st)
                for h in range(2):
                    ro_sb = outp.tile([P, 2, C], f32, tag="rosb", name=f"rosb{g2}_{h}")
                    nc.vector.tensor_copy(out=ro_sb, in_=dps[h])
                    nc.sync.dma_start(out=ro_view[:, g2 * 4 + 2 * h: g2 * 4 + 2 * h + 2, :], in_=ro_sb)

            # ---------- shared expert (I-shard of 512) ----------
            x_view = x_t.ap().rearrange("(k p) t -> p k t", p=P)
            for t in range(2):  # token chunks of 512
                xt_sb = xtp.tile([P, KH, 512], f32r, tag="xt", name=f"xt{t}")
                for k in range(KH):
                    nc.sync.dma_start(out=xt_sb[:, k, :], in_=x_view[:, k, t * 512:(t + 1) * 512])
                sgps = [psum.tile([P, 512], f32, tag="ps", name=f"sgps{t}_{m}") for m in range(4)]
                sups = [psum.tile([P, 512], f32, tag="ps", name=f"sups{t}_{m}") for m in range(4)]
                for k in range(KH):
                    wsg_b = wpool.tile([P, 512], f32r, tag="wblk", name=f"wsgb{t}_{k}")
                    nc.sync.dma_start(out=wsg_b, in_=wsg.ap()[k * P:(k + 1) * P, :])
                    wsu_b = wpool.tile([P, 512], f32r, tag="wblk", name=f"wsub{t}_{k}")
                    nc.sync.dma_start(out=wsu_b, in_=wsu.ap()[k * P:(k + 1) * P, :])
                    st = dict(start=(k == 0), stop=(k == KH - 1))
                    for m in range(4):
                        nc.tensor.matmul(sgps[m], wsg_b[:, m * P:(m + 1) * P], xt_sb[:, k, :], **st)
                    for m in range(4):
                        nc.tensor.matmul(sups[m], wsu_b[:, m * P:(m + 1) * P], xt_sb[:, k, :], **st)
                hs = hsp.tile([P, KSH, 512], f32r, tag="hs", name=f"hs{t}")
                for m in range(4):
                    hstmp = hbuf.tile([P, 512], f32, tag="hsb", name=f"hstmp{t}_{m}")
                    nc.scalar.activation(out=hstmp, in_=sgps[m], func=SILU)
                    nc.vector.tensor_tensor(hs[:, m, :], hstmp, sups[m], MULT)
                for m2 in range(MH):  # 16
                    sps = psum.tile([P, 512], f32, tag="ps", name=f"sps{t}_{m2}")
                    for k2 in range(KSH):
                        nc.tensor.matmul(sps, wsd_sb[:, k2, m2 * P:(m2 + 1) * P], hs[:, k2, :],
                                         start=(k2 == 0), stop=(k2 == KSH - 1))
                    sp_sb = outp.tile([P, 512], f32, tag="spsb", name=f"spsb{t}_{m2}")
                    nc.vector.tensor_copy(out=sp_sb, in_=sps)
                    nc.sync.dma_start(out=sp_view[:, m2, t * 512:(t + 1) * 512], in_=sp_sb)

    # fp32r matmuls are self-loading (no separate InstLdweights), so surplus
    # semaphore waits on a matmul break walrus codegen (1-wait limit). Split
    # them onto InstEventSemaphore carriers like bacc does before compiling.
    import bass_rust
    bass_rust.generate_event_semaphores(nc)
    return nc


def _get_bass(C):
    if C not in _BASS_CACHE:
        _BASS_CACHE[C] = _build_bass(C)
    return _BASS_CACHE[C]


def kernel(**inputs):
    global LAST_RESULT
    x = np.ascontiguousarray(np.asarray(inputs["x"], dtype=np.float32))
    w_router = np.asarray(inputs["w_router"], dtype=np.float32)
    ws_gate = np.asarray(inputs["ws_gate"], dtype=np.float32)
    ws_up = np.asarray(inputs["ws_up"], dtype=np.float32)
    ws_down = np.asarray(inputs["ws_down"], dtype=np.float32)
    we_gate = np.asarray(inputs["we_gate"], dtype=np.float32)
    we_up = np.asarray(inputs["we_up"], dtype=np.float32)
    we_down = np.asarray(inputs["we_down"], dtype=np.float32)

    # --- top-1 routing on host (tiny) ---
    logits = x @ w_router                      # [T, E]
    top = np.argmax(logits, axis=1)            # [T]
    tv = logits[np.arange(x.shape[0]), top]
    score = (1.0 / (1.0 + np.exp(-tv))).astype(np.float32)
    idxs = [np.nonzero(top == e)[0] for e in range(E)]
    maxn = max(len(i) for i in idxs)
    C = max(256, ((maxn + 127) // 128) * 128)

    nc = _get_bass(C)

    x_t = np.ascontiguousarray(x.T)            # [H, T]
    in_maps = []
    for e in range(E):
        idx = idxs[e]
        xe = np.zeros((C, H), np.float32)
        if len(idx):
            xe[:len(idx)] = x[idx] * score[idx, None]
        in_maps.append({
            "xe_t": np.ascontiguousarray(xe.T),
            "wg": np.ascontiguousarray(we_gate[e]),
            "wu": np.ascontiguousarray(we_up[e]),
            "wd": np.ascontiguousarray(we_down[e]),
            "x_t": x_t,
            "wsg": np.ascontiguousarray(ws_gate[:, e * ISH:(e + 1) * ISH]),
            "wsu": np.ascontiguousarray(ws_up[:, e * ISH:(e + 1) * ISH]),
            "wsd": np.ascontiguousarray(ws_down[e * ISH:(e + 1) * ISH, :]),
        })

    from concourse.bass_utils import run_bass_kernel_spmd
    res = run_bass_kernel_spmd(nc, in_maps, core_ids=list(range(E)))
    LAST_RESULT = res
    outs = res.results

    spT = outs[0]["sp_t"].copy()
    for e in range(1, E):
        spT += outs[e]["sp_t"]
    out = np.ascontiguousarray(spT.T)          # [T, H]
    for e in range(E):
        idx = idxs[e]
        if len(idx):
            out[idx] += outs[e]["ro_t"][:, :len(idx)].T
    return out



# revision 4
# speedup vs baseline: 1.8254x; 1.8254x over previous
"""Llama4-style MoE (T=1024, H=2048, I=4096, E=8, top-1) on 8 trn2 NeuronCores.

Sharding: expert-parallel. Core e owns expert e's weights plus a 1/8 I-shard
of the shared expert. Host computes top-1 routing (tiny [1024,8] matmul) and
dispatches each expert's tokens (scaled by the sigmoid router score, padded to
capacity C) to its core. Each core returns its expert's MLP output plus a
partial shared-expert output; host sums the partials and scatters the routed
rows back.

All device-side data is bf16 (weights are streamed once, so HBM traffic is the
roofline: ~63 MB/core ≈ 176 us at 358 GB/s; tensor time at C=144 is ~174 us —
balanced). Host pre-packs every tensor into [128 partitions, ...] layout with
contiguous per-partition lines so each weight DMA is a single 1-4 MB transfer.
Activations live in "transposed" space ([feature, token], feature on
partitions) so no on-chip transposes are needed. PSUM accumulates fp32.

Program order puts the shared expert first: its DMA footprint is small
(~11 MB) but its tensor work is large (~82 us), so the big routed weight
stream (~48 MB) flows underneath it.
"""

import numpy as np
import ml_dtypes

T, H, I, E = 1024, 2048, 4096, 8
P = 128
ISH = I // E          # 512  shared-expert I-shard per core
KH = H // P           # 16
MI = I // P           # 32
KSH = ISH // P        # 4
NT = 256              # shared-expert token chunk
NCH = T // NT         # 4 chunks

BF = ml_dtypes.bfloat16

_BASS_CACHE = {}
LAST_RESULT = None    # BassKernelResults of the most recent run (for test harness)
LAST_NC = None


def _pack_runs(C):
    """Split the 4 m-tiles of a 512-wide group into runs that each fit one
    2 KB PSUM bank ([P, q, C] fp32 with q*C <= 512)."""
    pack = max(1, 512 // C)
    runs = []
    left = 4
    while left:
        q = min(pack, left)
        runs.append(q)
        left -= q
    return runs


def _build_bass(C):
    import concourse.bass as bass
    import concourse.mybir as mybir
    import concourse.tile as tile

    assert C <= 512, f"routed capacity {C} > 512 unsupported"

    f32 = mybir.dt.float32
    bf16 = mybir.dt.bfloat16
    SILU = mybir.ActivationFunctionType.Silu
    MULT = mybir.AluOpType.mult

    nc = bass.Bass(trn_type="TRN2", name=f"moe_bf16_c{C}")

    # --- DRAM tensors, all host-packed to [128, ...] partition-major bf16 ---
    xe3 = nc.dram_tensor("xe3", [P, KH, C], bf16, kind="ExternalInput")
    wgu = nc.dram_tensor("wgu", [P, 8, 2, KH, 512], bf16, kind="ExternalInput")
    wd3 = nc.dram_tensor("wd3", [P, 4, MI, 512], bf16, kind="ExternalInput")
    x3 = nc.dram_tensor("x3", [P, NCH, KH, NT], bf16, kind="ExternalInput")
    wsg3 = nc.dram_tensor("wsg3", [P, KH, ISH], bf16, kind="ExternalInput")
    wsu3 = nc.dram_tensor("wsu3", [P, KH, ISH], bf16, kind="ExternalInput")
    wsd3 = nc.dram_tensor("wsd3", [P, KSH, H], bf16, kind="ExternalInput")
    ro = nc.dram_tensor("ro", [P, KH, C], bf16, kind="ExternalOutput")
    sp = nc.dram_tensor("sp", [P, NCH, KH, NT], bf16, kind="ExternalOutput")

    runs = _pack_runs(C)

    with tile.TileContext(nc) as tc:
        from contextlib import ExitStack

        with ExitStack() as ctx:
            const = ctx.enter_context(tc.tile_pool(name="const", bufs=1))
            xpool = ctx.enter_context(tc.tile_pool(name="xpool", bufs=2))
            wpool = ctx.enter_context(tc.tile_pool(name="wpool", bufs=2))
            hsp = ctx.enter_context(tc.tile_pool(name="hsp", bufs=2))
            hbuf = ctx.enter_context(tc.tile_pool(name="hbuf", bufs=2))
            outp = ctx.enter_context(tc.tile_pool(name="outp", bufs=2))
            psum = ctx.enter_context(tc.tile_pool(name="psum", bufs=8, space="PSUM"))

            # --- resident loads (shared-expert working set + routed tokens) ---
            xeT = const.tile([P, KH, C], bf16)
            nc.sync.dma_start(out=xeT, in_=xe3.ap())
            wsg_sb = const.tile([P, KH, ISH], bf16)
            nc.sync.dma_start(out=wsg_sb, in_=wsg3.ap())
            wsu_sb = const.tile([P, KH, ISH], bf16)
            nc.sync.dma_start(out=wsu_sb, in_=wsu3.ap())
            wsd_sb = const.tile([P, KSH, H], bf16)
            nc.sync.dma_start(out=wsd_sb, in_=wsd3.ap())

            # ---------- shared expert (I-shard of 512), token chunks ----------
            for t in range(NCH):
                xt = xpool.tile([P, KH, NT], bf16, tag="xt", name=f"xt{t}")
                nc.sync.dma_start(out=xt, in_=x3.ap()[:, t])
                sg = [psum.tile([P, NT], f32, tag="ps", name=f"sg{t}_{m}")
                      for m in range(KSH)]
                su = [psum.tile([P, NT], f32, tag="ps", name=f"su{t}_{m}")
                      for m in range(KSH)]
                for k in range(KH):
                    st = dict(start=(k == 0), stop=(k == KH - 1))
                    for m in range(KSH):
                        nc.tensor.matmul(sg[m], wsg_sb[:, k, m * P:(m + 1) * P],
                                         xt[:, k, :], **st)
                    for m in range(KSH):
                        nc.tensor.matmul(su[m], wsu_sb[:, k, m * P:(m + 1) * P],
                                         xt[:, k, :], **st)
                hs = hsp.tile([P, KSH, NT], bf16, tag="hs", name=f"hs{t}")
                for m in range(KSH):
                    htmp = hbuf.tile([P, NT], bf16, tag="hsb", name=f"htmp{t}_{m}")
                    nc.scalar.activation(out=htmp, in_=sg[m], func=SILU)
                    nc.vector.tensor_tensor(hs[:, m, :], htmp, su[m], MULT)
                for mg in range(4):  # 16 H-tiles in groups of 4
                    sp_sb = outp.tile([P, 4, NT], bf16, tag="spsb",
                                      name=f"spsb{t}_{mg}")
                    for m2l in range(4):
                        m2 = mg * 4 + m2l
                        ps = psum.tile([P, NT], f32, tag="ps", name=f"sps{t}_{m2}")
                        for k2 in range(KSH):
                            nc.tensor.matmul(ps, wsd_sb[:, k2, m2 * P:(m2 + 1) * P],
                                             hs[:, k2, :],
                                             start=(k2 == 0), stop=(k2 == KSH - 1))
                        nc.vector.tensor_copy(out=sp_sb[:, m2l, :], in_=ps)
                    nc.sync.dma_start(out=sp.ap()[:, t, mg * 4:(mg + 1) * 4, :],
                                      in_=sp_sb)

            # ---------- routed expert: gate/up -> hT ----------
            hTp = ctx.enter_context(tc.tile_pool(name="hTp", bufs=1))
            hT = hTp.tile([P, MI, C], bf16)

            for g in range(8):  # 8 groups of 512 I-columns
                wgB = wpool.tile([P, KH, 512], bf16, tag="wg", name=f"wgB{g}")
                nc.sync.dma_start(out=wgB, in_=wgu.ap()[:, g, 0])
                wuB = wpool.tile([P, KH, 512], bf16, tag="wu", name=f"wuB{g}")
                nc.sync.dma_start(out=wuB, in_=wgu.ap()[:, g, 1])
                gps, ups, mmap = [], [], []
                for ri, q in enumerate(runs):
                    gps.append(psum.tile([P, q, C], f32, tag="ps",
                                         name=f"gps{g}_{ri}"))
                    ups.append(psum.tile([P, q, C], f32, tag="ps",
                                         name=f"ups{g}_{ri}"))
                    for j in range(q):
                        mmap.append((ri, j))
                for k in range(KH):
                    for mi in range(4):
                        ri, j = mmap[mi]
                        # start only on the first write to each PSUM bank:
                        # start=True clears the whole bank's has_written bits.
                        st = dict(start=(k == 0 and j == 0), stop=(k == KH - 1))
                        nc.tensor.matmul(gps[ri][:, j, :],
                                         wgB[:, k, mi * P:(mi + 1) * P],
                                         xeT[:, k, :], **st)
                    for mi in range(4):
                        ri, j = mmap[mi]
                        st = dict(start=(k == 0 and j == 0), stop=(k == KH - 1))
                        nc.tensor.matmul(ups[ri][:, j, :],
                                         wuB[:, k, mi * P:(mi + 1) * P],
                                         xeT[:, k, :], **st)
                off = 0
                for ri, q in enumerate(runs):
                    h_sb = hbuf.tile([P, q, C], bf16, tag="hrb",
                                     name=f"hrb{g}_{ri}")
                    nc.scalar.activation(out=h_sb, in_=gps[ri], func=SILU)
                    nc.vector.tensor_tensor(hT[:, g * 4 + off:g * 4 + off + q, :],
                                            h_sb, ups[ri], MULT)
                    off += q

            # ---------- routed expert: down -> ro ----------
            for g2 in range(4):  # 4 groups of 512 H-columns
                wdBs = []
                for half in range(2):
                    wdB = wpool.tile([P, KH, 512], bf16, tag="wd",
                                     name=f"wdB{g2}_{half}")
                    nc.sync.dma_start(
                        out=wdB, in_=wd3.ap()[:, g2, half * KH:(half + 1) * KH, :])
                    wdBs.append(wdB)
                dps, mmap = [], []
                for ri, q in enumerate(runs):
                    dps.append(psum.tile([P, q, C], f32, tag="ps",
                                         name=f"dps{g2}_{ri}"))
                    for j in range(q):
                        mmap.append((ri, j))
                for k2 in range(MI):
                    wb = wdBs[k2 // KH]
                    kk = k2 % KH
                    for mi in range(4):
                        ri, j = mmap[mi]
                        st = dict(start=(k2 == 0 and j == 0), stop=(k2 == MI - 1))
                        nc.tensor.matmul(dps[ri][:, j, :],
                                         wb[:, kk, mi * P:(mi + 1) * P],
                                         hT[:, k2, :], **st)
                ro_sb = outp.tile([P, 4, C], bf16, tag="rosb", name=f"rosb{g2}")
                off = 0
                for ri, q in enumerate(runs):
                    nc.vector.tensor_copy(out=ro_sb[:, off:off + q, :], in_=dps[ri])
                    off += q
                nc.sync.dma_start(out=ro.ap()[:, g2 * 4:(g2 + 1) * 4, :], in_=ro_sb)

    # Split surplus semaphore waits onto InstEventSemaphore carriers (walrus
    # has a 1-wait limit per instruction).
    import bass_rust
    bass_rust.generate_event_semaphores(nc)
    return nc


def _get_bass(C):
    if C not in _BASS_CACHE:
        _BASS_CACHE[C] = _build_bass(C)
    return _BASS_CACHE[C]


def kernel(**inputs):
    global LAST_RESULT, LAST_NC
    x = np.ascontiguousarray(np.asarray(inputs["x"], dtype=np.float32))
    w_router = np.asarray(inputs["w_router"], dtype=np.float32)
    ws_gate = np.asarray(inputs["ws_gate"], dtype=np.float32)
    ws_up = np.asarray(inputs["ws_up"], dtype=np.float32)
    ws_down = np.asarray(inputs["ws_down"], dtype=np.float32)
    we_gate = np.asarray(inputs["we_gate"], dtype=np.float32)
    we_up = np.asarray(inputs["we_up"], dtype=np.float32)
    we_down = np.asarray(inputs["we_down"], dtype=np.float32)

    # --- top-1 routing on host (tiny) ---
    logits = x @ w_router                      # [T, E]
    top = np.argmax(logits, axis=1)            # [T]
    tv = logits[np.arange(T), top]
    score = (1.0 / (1.0 + np.exp(-tv))).astype(np.float32)
    idxs = [np.nonzero(top == e)[0] for e in range(E)]
    maxn = max(len(i) for i in idxs)
    C = max(P, ((maxn + 15) // 16) * 16)

    nc = _get_bass(C)
    LAST_NC = nc

    # x3[p, t, k, j] = x[t*NT + j, k*128 + p]
    x3 = x.reshape(NCH, NT, KH, P).transpose(3, 0, 2, 1).astype(BF)

    in_maps = []
    for e in range(E):
        idx = idxs[e]
        xe = np.zeros((C, H), np.float32)
        if len(idx):
            xe[:len(idx)] = x[idx] * score[idx, None]
        # xe3[p, k, c] = xe[c, k*128 + p]
        xe3 = xe.reshape(C, KH, P).transpose(2, 1, 0).astype(BF)

        # wgu[p, g, w, k, j] = we_{gate,up}[e][k*128 + p, g*512 + j]
        wgu = np.empty((P, 8, 2, KH, 512), BF)
        wgu[:, :, 0] = we_gate[e].reshape(KH, P, 8, 512).transpose(1, 2, 0, 3)
        wgu[:, :, 1] = we_up[e].reshape(KH, P, 8, 512).transpose(1, 2, 0, 3)
        # wd3[p, g2, k2, j] = we_down[e][k2*128 + p, g2*512 + j]
        wd3 = we_down[e].reshape(MI, P, 4, 512).transpose(1, 2, 0, 3).astype(BF)

        # shared-expert shard for this core
        wsg3 = ws_gate[:, e * ISH:(e + 1) * ISH].reshape(KH, P, ISH) \
            .transpose(1, 0, 2).astype(BF)
        wsu3 = ws_up[:, e * ISH:(e + 1) * ISH].reshape(KH, P, ISH) \
            .transpose(1, 0, 2).astype(BF)
        wsd3 = ws_down[e * ISH:(e + 1) * ISH].reshape(KSH, P, H) \
            .transpose(1, 0, 2).astype(BF)

        in_maps.append({
            "xe3": xe3, "wgu": wgu, "wd3": wd3, "x3": x3,
            "wsg3": wsg3, "wsu3": wsu3, "wsd3": wsd3,
        })

    from concourse.bass_utils import run_bass_kernel_spmd
    res = run_bass_kernel_spmd(nc, in_maps, core_ids=list(range(E)))
    LAST_RESULT = res
    outs = res.results

    # shared partials: sp[p, t, m2, j] -> [token, h], summed over cores
    spsum = np.zeros((P, NCH, KH, NT), np.float32)
    for e in range(E):
        spsum += outs[e]["sp"].astype(np.float32)
    out = np.ascontiguousarray(
        spsum.transpose(1, 3, 2, 0).reshape(T, H))

    # routed: ro[p, m, c] -> [c, h], scatter back by token index
    for e in range(E):
        idx = idxs[e]
        if len(idx):
            roe = outs[e]["ro"].astype(np.float32)
            out[idx] += roe.transpose(2, 1, 0).reshape(C, H)[:len(idx)]
    return out


# revision 5
# speedup vs baseline: 2.0202x; 1.1067x over previous
"""Llama4-style MoE (T=1024, H=2048, I=4096, E=8, top-1) on 8 trn2 NeuronCores.

Sharding: expert-parallel. Core e owns expert e's weights plus a 1/8 I-shard
of the shared expert. Host computes top-1 routing (tiny [1024,8] matmul) and
dispatches each expert's tokens (scaled by the sigmoid router score, padded to
capacity C) to its core. Each core returns its expert's MLP output plus a
partial shared-expert output; host sums the partials and scatters the routed
rows back.

All device-side data is bf16 (weights are streamed once, so HBM traffic is the
roofline: ~63 MB/core ≈ 176 us at 358 GB/s; tensor time at C=144 is ~174 us —
balanced). Host pre-packs every tensor into [128 partitions, ...] layout with
contiguous per-partition lines so each weight DMA is a single 1-4 MB transfer.
Activations live in "transposed" space ([feature, token], feature on
partitions) so no on-chip transposes are needed. PSUM accumulates fp32.

Program order puts the shared expert first: its DMA footprint is small
(~11 MB) but its tensor work is large (~82 us), so the big routed weight
stream (~48 MB) flows underneath it.
"""

import numpy as np
import ml_dtypes

T, H, I, E = 1024, 2048, 4096, 8
P = 128
ISH = I // E          # 512  shared-expert I-shard per core
KH = H // P           # 16
MI = I // P           # 32
KSH = ISH // P        # 4
NT = 256              # shared-expert token chunk
NCH = T // NT         # 4 chunks

BF = ml_dtypes.bfloat16

_BASS_CACHE = {}
LAST_RESULT = None    # BassKernelResults of the most recent run (for test harness)
LAST_NC = None


def _pack_runs(C):
    """Split the 4 m-tiles of a 512-wide group into runs that each fit one
    2 KB PSUM bank ([P, q, C] fp32 with q*C <= 512)."""
    pack = max(1, 512 // C)
    runs = []
    left = 4
    while left:
        q = min(pack, left)
        runs.append(q)
        left -= q
    return runs


def _build_bass(C):
    import concourse.bass as bass
    import concourse.mybir as mybir
    import concourse.tile as tile

    assert C <= 512, f"routed capacity {C} > 512 unsupported"

    f32 = mybir.dt.float32
    bf16 = mybir.dt.bfloat16
    SILU = mybir.ActivationFunctionType.Silu
    MULT = mybir.AluOpType.mult

    nc = bass.Bass(trn_type="TRN2", name=f"moe_bf16_c{C}")

    # --- DRAM tensors, all host-packed to [128, ...] partition-major bf16 ---
    xe3 = nc.dram_tensor("xe3", [P, KH, C], bf16, kind="ExternalInput")
    wgu = nc.dram_tensor("wgu", [P, 8, 2, KH, 512], bf16, kind="ExternalInput")
    wd3 = nc.dram_tensor("wd3", [P, 4, MI, 512], bf16, kind="ExternalInput")
    x3 = nc.dram_tensor("x3", [P, NCH, KH, NT], bf16, kind="ExternalInput")
    wsg3 = nc.dram_tensor("wsg3", [P, KH, ISH], bf16, kind="ExternalInput")
    wsu3 = nc.dram_tensor("wsu3", [P, KH, ISH], bf16, kind="ExternalInput")
    wsd3 = nc.dram_tensor("wsd3", [P, KSH, H], bf16, kind="ExternalInput")
    ro = nc.dram_tensor("ro", [P, KH, C], bf16, kind="ExternalOutput")
    sp = nc.dram_tensor("sp", [P, NCH, KH, NT], bf16, kind="ExternalOutput")

    runs = _pack_runs(C)

    with tile.TileContext(nc) as tc:
        from contextlib import ExitStack

        with ExitStack() as ctx:
            const = ctx.enter_context(tc.tile_pool(name="const", bufs=1))
            xpool = ctx.enter_context(tc.tile_pool(name="xpool", bufs=2))
            wpool = ctx.enter_context(tc.tile_pool(name="wpool", bufs=5))
            hsp = ctx.enter_context(tc.tile_pool(name="hsp", bufs=2))
            hbuf = ctx.enter_context(tc.tile_pool(name="hbuf", bufs=2))
            outp = ctx.enter_context(tc.tile_pool(name="outp", bufs=2))
            psum = ctx.enter_context(tc.tile_pool(name="psum", bufs=8, space="PSUM"))
            hTp = ctx.enter_context(tc.tile_pool(name="hTp", bufs=1))

            # DMA engine split: weights on the SP HWDGE ring (nc.sync),
            # input/activation loads on the ACT HWDGE ring (nc.scalar),
            # outputs on SWDGE (nc.gpsimd) — three independent rings so
            # fixed costs overlap.

            # --- resident loads ---
            xeT = const.tile([P, KH, C], bf16)
            nc.scalar.dma_start(out=xeT, in_=xe3.ap())
            wsg_sb = const.tile([P, KH, ISH], bf16)
            nc.scalar.dma_start(out=wsg_sb, in_=wsg3.ap())
            wsu_sb = const.tile([P, KH, ISH], bf16)
            nc.scalar.dma_start(out=wsu_sb, in_=wsu3.ap())
            wsd_sb = const.tile([P, KSH, H], bf16)
            nc.scalar.dma_start(out=wsd_sb, in_=wsd3.ap())

            hT = hTp.tile([P, MI, C], bf16)
            ro_sb = const.tile([P, KH, C], bf16, name="ro_sb")

            def routed_gu_group(g):
                """Routed expert gate/up for one 512-wide I group -> hT."""
                wgB = wpool.tile([P, KH, 512], bf16, tag="w", name=f"wgB{g}")
                nc.sync.dma_start(out=wgB, in_=wgu.ap()[:, g, 0])
                wuB = wpool.tile([P, KH, 512], bf16, tag="w", name=f"wuB{g}")
                nc.sync.dma_start(out=wuB, in_=wgu.ap()[:, g, 1])
                gps, ups, mmap = [], [], []
                for ri, q in enumerate(runs):
                    gps.append(psum.tile([P, q, C], f32, tag="ps",
                                         name=f"gps{g}_{ri}"))
                    ups.append(psum.tile([P, q, C], f32, tag="ps",
                                         name=f"ups{g}_{ri}"))
                    for j in range(q):
                        mmap.append((ri, j))
                # gate over all k first, then up: wuB may still be in flight
                # while the gate matmuls run.
                for ps_tiles, wB in ((gps, wgB), (ups, wuB)):
                    for k in range(KH):
                        for mi in range(4):
                            ri, j = mmap[mi]
                            # start only on the first write to each PSUM bank:
                            # start=True clears the whole bank's has_written.
                            st = dict(start=(k == 0 and j == 0),
                                      stop=(k == KH - 1))
                            nc.tensor.matmul(ps_tiles[ri][:, j, :],
                                             wB[:, k, mi * P:(mi + 1) * P],
                                             xeT[:, k, :], **st)
                off = 0
                for ri, q in enumerate(runs):
                    h_sb = hbuf.tile([P, q, C], bf16, tag="hrb",
                                     name=f"hrb{g}_{ri}")
                    nc.scalar.activation(out=h_sb, in_=gps[ri], func=SILU)
                    nc.vector.tensor_tensor(hT[:, g * 4 + off:g * 4 + off + q, :],
                                            h_sb, ups[ri], MULT)
                    off += q

            def shared_chunk(t):
                """Shared expert (I-shard) for one 256-token chunk -> sp."""
                xt = xpool.tile([P, KH, NT], bf16, tag="xt", name=f"xt{t}")
                nc.scalar.dma_start(out=xt, in_=x3.ap()[:, t])
                # pack two [P, NT] fp32 accumulators per PSUM bank
                sg = [psum.tile([P, 2, NT], f32, tag="ps", name=f"sg{t}_{r}")
                      for r in range(2)]
                su = [psum.tile([P, 2, NT], f32, tag="ps", name=f"su{t}_{r}")
                      for r in range(2)]
                for ps_tiles, wB in ((sg, wsg_sb), (su, wsu_sb)):
                    for k in range(KH):
                        for m in range(KSH):
                            st = dict(start=(k == 0 and m % 2 == 0),
                                      stop=(k == KH - 1))
                            nc.tensor.matmul(ps_tiles[m // 2][:, m % 2, :],
                                             wB[:, k, m * P:(m + 1) * P],
                                             xt[:, k, :], **st)
                hs = hsp.tile([P, KSH, NT], bf16, tag="hs", name=f"hs{t}")
                for r in range(2):
                    htmp = hbuf.tile([P, 2, NT], bf16, tag="hsb",
                                     name=f"htmp{t}_{r}")
                    nc.scalar.activation(out=htmp, in_=sg[r], func=SILU)
                    nc.vector.tensor_tensor(hs[:, 2 * r:2 * r + 2, :], htmp,
                                            su[r], MULT)
                sp_sb = outp.tile([P, KH, NT], bf16, tag="spsb", name=f"spsb{t}")
                for m2 in range(KH):
                    ps = psum.tile([P, NT], f32, tag="ps", name=f"sps{t}_{m2}")
                    for k2 in range(KSH):
                        nc.tensor.matmul(ps, wsd_sb[:, k2, m2 * P:(m2 + 1) * P],
                                         hs[:, k2, :],
                                         start=(k2 == 0), stop=(k2 == KSH - 1))
                    nc.vector.tensor_copy(out=sp_sb[:, m2, :], in_=ps)
                nc.gpsimd.dma_start(out=sp.ap()[:, t], in_=sp_sb)

            def routed_down_group(g2):
                """Routed expert down-proj for one 512-wide H group -> ro_sb."""
                wdBs = []
                for half in range(2):
                    wdB = wpool.tile([P, KH, 512], bf16, tag="w",
                                     name=f"wdB{g2}_{half}")
                    nc.sync.dma_start(
                        out=wdB,
                        in_=wd3.ap()[:, g2, half * KH:(half + 1) * KH, :])
                    wdBs.append(wdB)
                dps, mmap = [], []
                for ri, q in enumerate(runs):
                    dps.append(psum.tile([P, q, C], f32, tag="ps",
                                         name=f"dps{g2}_{ri}"))
                    for j in range(q):
                        mmap.append((ri, j))
                for k2 in range(MI):
                    wb = wdBs[k2 // KH]
                    kk = k2 % KH
                    for mi in range(4):
                        ri, j = mmap[mi]
                        st = dict(start=(k2 == 0 and j == 0),
                                  stop=(k2 == MI - 1))
                        nc.tensor.matmul(dps[ri][:, j, :],
                                         wb[:, kk, mi * P:(mi + 1) * P],
                                         hT[:, k2, :], **st)
                off = 0
                for ri, q in enumerate(runs):
                    nc.vector.tensor_copy(out=ro_sb[:, g2 * 4 + off:
                                                    g2 * 4 + off + q, :],
                                          in_=dps[ri])
                    off += q

            # Interleave: routed groups are DMA-heavy (4 MB / 7.7 us tensor),
            # shared chunks are tensor-heavy (3 MB resident / 20.5 us tensor).
            # Spreading the chunks between routed groups keeps both the DMA
            # queue and the PE busy end-to-end.
            routed_gu_group(0)
            shared_chunk(0)
            routed_gu_group(1)
            routed_gu_group(2)
            shared_chunk(1)
            routed_gu_group(3)
            routed_gu_group(4)
            shared_chunk(2)
            routed_gu_group(5)
            routed_gu_group(6)
            shared_chunk(3)
            routed_gu_group(7)
            for g2 in range(4):
                routed_down_group(g2)
            nc.gpsimd.dma_start(out=ro.ap(), in_=ro_sb)

    # Split surplus semaphore waits onto InstEventSemaphore carriers (walrus
    # has a 1-wait limit per instruction).
    import bass_rust
    bass_rust.generate_event_semaphores(nc)
    return nc


def _get_bass(C):
    if C not in _BASS_CACHE:
        _BASS_CACHE[C] = _build_bass(C)
    return _BASS_CACHE[C]


def kernel(**inputs):
    global LAST_RESULT, LAST_NC
    x = np.ascontiguousarray(np.asarray(inputs["x"], dtype=np.float32))
    w_router = np.asarray(inputs["w_router"], dtype=np.float32)
    ws_gate = np.asarray(inputs["ws_gate"], dtype=np.float32)
    ws_up = np.asarray(inputs["ws_up"], dtype=np.float32)
    ws_down = np.asarray(inputs["ws_down"], dtype=np.float32)
    we_gate = np.asarray(inputs["we_gate"], dtype=np.float32)
    we_up = np.asarray(inputs["we_up"], dtype=np.float32)
    we_down = np.asarray(inputs["we_down"], dtype=np.float32)

    # --- top-1 routing on host (tiny) ---
    logits = x @ w_router                      # [T, E]
    top = np.argmax(logits, axis=1)            # [T]
    tv = logits[np.arange(T), top]
    score = (1.0 / (1.0 + np.exp(-tv))).astype(np.float32)
    idxs = [np.nonzero(top == e)[0] for e in range(E)]
    maxn = max(len(i) for i in idxs)
    C = max(P, ((maxn + 15) // 16) * 16)

    nc = _get_bass(C)
    LAST_NC = nc

    # x3[p, t, k, j] = x[t*NT + j, k*128 + p]
    x3 = x.reshape(NCH, NT, KH, P).transpose(3, 0, 2, 1).astype(BF)

    in_maps = []
    for e in range(E):
        idx = idxs[e]
        xe = np.zeros((C, H), np.float32)
        if len(idx):
            xe[:len(idx)] = x[idx] * score[idx, None]
        # xe3[p, k, c] = xe[c, k*128 + p]
        xe3 = xe.reshape(C, KH, P).transpose(2, 1, 0).astype(BF)

        # wgu[p, g, w, k, j] = we_{gate,up}[e][k*128 + p, g*512 + j]
        wgu = np.empty((P, 8, 2, KH, 512), BF)
        wgu[:, :, 0] = we_gate[e].reshape(KH, P, 8, 512).transpose(1, 2, 0, 3)
        wgu[:, :, 1] = we_up[e].reshape(KH, P, 8, 512).transpose(1, 2, 0, 3)
        # wd3[p, g2, k2, j] = we_down[e][k2*128 + p, g2*512 + j]
        wd3 = we_down[e].reshape(MI, P, 4, 512).transpose(1, 2, 0, 3).astype(BF)

        # shared-expert shard for this core
        wsg3 = ws_gate[:, e * ISH:(e + 1) * ISH].reshape(KH, P, ISH) \
            .transpose(1, 0, 2).astype(BF)
        wsu3 = ws_up[:, e * ISH:(e + 1) * ISH].reshape(KH, P, ISH) \
            .transpose(1, 0, 2).astype(BF)
        wsd3 = ws_down[e * ISH:(e + 1) * ISH].reshape(KSH, P, H) \
            .transpose(1, 0, 2).astype(BF)

        in_maps.append({
            "xe3": xe3, "wgu": wgu, "wd3": wd3, "x3": x3,
            "wsg3": wsg3, "wsu3": wsu3, "wsd3": wsd3,
        })

    from concourse.bass_utils import run_bass_kernel_spmd
    res = run_bass_kernel_spmd(nc, in_maps, core_ids=list(range(E)))
    LAST_RESULT = res
    outs = res.results

    # shared partials: sp[p, t, m2, j] -> [token, h], summed over cores
    spsum = np.zeros((P, NCH, KH, NT), np.float32)
    for e in range(E):
        spsum += outs[e]["sp"].astype(np.float32)
    out = np.ascontiguousarray(
        spsum.transpose(1, 3, 2, 0).reshape(T, H))

    # routed: ro[p, m, c] -> [c, h], scatter back by token index
    for e in range(E):
        idx = idxs[e]
        if len(idx):
            roe = outs[e]["ro"].astype(np.float32)
            out[idx] += roe.transpose(2, 1, 0).reshape(C, H)[:len(idx)]
    return out


# revision 8
# speedup vs baseline: 2.0635x; 1.0214x over previous
"""Llama4-style MoE (T=1024, H=2048, I=4096, E=8, top-1) on 8 trn2 NeuronCores.

Sharding: expert-parallel. Core e owns expert e's weights plus a 1/8 I-shard
of the shared expert. Host computes top-1 routing (tiny [1024,8] matmul) and
dispatches each expert's tokens (scaled by the sigmoid router score, padded to
capacity C) to its core. Each core returns its expert's MLP output plus a
partial shared-expert output; host sums the partials and scatters the routed
rows back.

All device-side data is bf16 (weights are streamed once, so HBM traffic is the
roofline: ~63 MB/core ≈ 176 us at 358 GB/s; tensor time at C=144 is ~174 us —
balanced). Host pre-packs every tensor into [128 partitions, ...] layout with
contiguous per-partition lines so each weight DMA is a single 1-4 MB transfer.
Activations live in "transposed" space ([feature, token], feature on
partitions) so no on-chip transposes are needed. PSUM accumulates fp32.

Program order puts the shared expert first: its DMA footprint is small
(~11 MB) but its tensor work is large (~82 us), so the big routed weight
stream (~48 MB) flows underneath it.
"""

import numpy as np
import ml_dtypes

T, H, I, E = 1024, 2048, 4096, 8
P = 128
ISH = I // E          # 512  shared-expert I-shard per core
KH = H // P           # 16
MI = I // P           # 32
KSH = ISH // P        # 4
NT = 256              # shared-expert token chunk
NCH = T // NT         # 4 chunks

BF = ml_dtypes.bfloat16

_BASS_CACHE = {}
LAST_RESULT = None    # BassKernelResults of the most recent run (for test harness)
LAST_NC = None


def _pack_runs(C):
    """Split the 4 m-tiles of a 512-wide group into runs that each fit one
    2 KB PSUM bank ([P, q, C] fp32 with q*C <= 512)."""
    pack = max(1, 512 // C)
    runs = []
    left = 4
    while left:
        q = min(pack, left)
        runs.append(q)
        left -= q
    return runs


def _build_bass(C):
    import concourse.bass as bass
    import concourse.mybir as mybir
    import concourse.tile as tile

    assert C <= 512, f"routed capacity {C} > 512 unsupported"

    f32 = mybir.dt.float32
    bf16 = mybir.dt.bfloat16
    SILU = mybir.ActivationFunctionType.Silu
    MULT = mybir.AluOpType.mult

    nc = bass.Bass(trn_type="TRN2", name=f"moe_bf16_c{C}")

    # --- DRAM tensors, all host-packed to [128, ...] partition-major bf16 ---
    xe3 = nc.dram_tensor("xe3", [P, KH, C], bf16, kind="ExternalInput")
    wgu = nc.dram_tensor("wgu", [P, 8, 2, KH, 512], bf16, kind="ExternalInput")
    wd3 = nc.dram_tensor("wd3", [P, 4, MI, 512], bf16, kind="ExternalInput")
    x3 = nc.dram_tensor("x3", [P, NCH, KH, NT], bf16, kind="ExternalInput")
    wsg3 = nc.dram_tensor("wsg3", [P, KH, ISH], bf16, kind="ExternalInput")
    wsu3 = nc.dram_tensor("wsu3", [P, KH, ISH], bf16, kind="ExternalInput")
    wsd3 = nc.dram_tensor("wsd3", [P, KSH, H], bf16, kind="ExternalInput")
    ro = nc.dram_tensor("ro", [P, KH, C], bf16, kind="ExternalOutput")
    sp = nc.dram_tensor("sp", [P, NCH, KH, NT], bf16, kind="ExternalOutput")

    runs = _pack_runs(C)

    with tile.TileContext(nc) as tc:
        from contextlib import ExitStack

        with ExitStack() as ctx:
            const = ctx.enter_context(tc.tile_pool(name="const", bufs=1))
            xpool = ctx.enter_context(tc.tile_pool(name="xpool", bufs=2))
            wpool = ctx.enter_context(tc.tile_pool(name="wpool", bufs=3))
            wdpool = ctx.enter_context(tc.tile_pool(name="wdpool", bufs=2))
            hsp = ctx.enter_context(tc.tile_pool(name="hsp", bufs=2))
            hbuf = ctx.enter_context(tc.tile_pool(name="hbuf", bufs=2))
            outp = ctx.enter_context(tc.tile_pool(name="outp", bufs=2))
            psum = ctx.enter_context(tc.tile_pool(name="psum", bufs=8, space="PSUM"))
            hTp = ctx.enter_context(tc.tile_pool(name="hTp", bufs=1))

            # DMA ring split: gate/up weight stream on the SP HWDGE ring
            # (nc.sync, ~32 MB), input loads + down-proj weight stream on the
            # ACT HWDGE ring (nc.scalar, ~26 MB), outputs on SWDGE
            # (nc.gpsimd, ~4.6 MB) — three independent rings so fixed costs
            # overlap and aggregate BW approaches the HBM limit.

            # --- early input loads, in tensor-consumption order ---
            xeT = const.tile([P, KH, C], bf16)
            nc.scalar.dma_start(out=xeT, in_=xe3.ap())
            xts = [None] * NCH

            def load_x(t):
                xts[t] = xpool.tile([P, KH, NT], bf16, tag="xt", name=f"xt{t}")
                nc.scalar.dma_start(out=xts[t], in_=x3.ap()[:, t])

            load_x(0)
            wsg_sb = const.tile([P, KH, ISH], bf16)
            nc.scalar.dma_start(out=wsg_sb, in_=wsg3.ap())
            wsu_sb = const.tile([P, KH, ISH], bf16)
            nc.scalar.dma_start(out=wsu_sb, in_=wsu3.ap())
            load_x(1)
            wsd_sb = const.tile([P, KSH, H], bf16)
            nc.scalar.dma_start(out=wsd_sb, in_=wsd3.ap())

            hT = hTp.tile([P, MI, C], bf16)
            ro_sb = const.tile([P, KH, C], bf16, name="ro_sb")

            def routed_gu_group(g):
                """Routed expert gate/up for one 512-wide I group -> hT."""
                wgB = wpool.tile([P, KH, 512], bf16, tag="w", name=f"wgB{g}")
                nc.sync.dma_start(out=wgB, in_=wgu.ap()[:, g, 0])
                wuB = wpool.tile([P, KH, 512], bf16, tag="w", name=f"wuB{g}")
                nc.sync.dma_start(out=wuB, in_=wgu.ap()[:, g, 1])
                gps, ups, mmap = [], [], []
                for ri, q in enumerate(runs):
                    gps.append(psum.tile([P, q, C], f32, tag="ps",
                                         name=f"gps{g}_{ri}"))
                    ups.append(psum.tile([P, q, C], f32, tag="ps",
                                         name=f"ups{g}_{ri}"))
                    for j in range(q):
                        mmap.append((ri, j))
                # gate over all k first, then up: wuB may still be in flight
                # while the gate matmuls run.
                for ps_tiles, wB in ((gps, wgB), (ups, wuB)):
                    for k in range(KH):
                        for mi in range(4):
                            ri, j = mmap[mi]
                            # start only on the first write to each PSUM bank:
                            # start=True clears the whole bank's has_written.
                            st = dict(start=(k == 0 and j == 0),
                                      stop=(k == KH - 1))
                            nc.tensor.matmul(ps_tiles[ri][:, j, :],
                                             wB[:, k, mi * P:(mi + 1) * P],
                                             xeT[:, k, :], **st)
                off = 0
                for ri, q in enumerate(runs):
                    h_sb = hbuf.tile([P, q, C], bf16, tag="hrb",
                                     name=f"hrb{g}_{ri}")
                    nc.scalar.activation(out=h_sb, in_=gps[ri], func=SILU)
                    nc.vector.tensor_tensor(hT[:, g * 4 + off:g * 4 + off + q, :],
                                            h_sb, ups[ri], MULT)
                    off += q

            hss = [None] * NCH

            def shared_gu(t):
                """Shared expert gate/up for one 256-token chunk -> hs[t]."""
                xt = xts[t]
                # pack two [P, NT] fp32 accumulators per PSUM bank
                sg = [psum.tile([P, 2, NT], f32, tag="ps", name=f"sg{t}_{r}")
                      for r in range(2)]
                su = [psum.tile([P, 2, NT], f32, tag="ps", name=f"su{t}_{r}")
                      for r in range(2)]
                for ps_tiles, wB in ((sg, wsg_sb), (su, wsu_sb)):
                    for k in range(KH):
                        for m in range(KSH):
                            st = dict(start=(k == 0 and m % 2 == 0),
                                      stop=(k == KH - 1))
                            nc.tensor.matmul(ps_tiles[m // 2][:, m % 2, :],
                                             wB[:, k, m * P:(m + 1) * P],
                                             xt[:, k, :], **st)
                hs = hsp.tile([P, KSH, NT], bf16, tag="hs", name=f"hs{t}")
                hss[t] = hs
                for r in range(2):
                    htmp = hbuf.tile([P, 2, NT], bf16, tag="hsb",
                                     name=f"htmp{t}_{r}")
                    nc.scalar.activation(out=htmp, in_=sg[r], func=SILU)
                    nc.vector.tensor_tensor(hs[:, 2 * r:2 * r + 2, :], htmp,
                                            su[r], MULT)

            def shared_down(t):
                """Shared expert down-proj for chunk t -> sp."""
                hs = hss[t]
                sp_sb = outp.tile([P, KH, NT], bf16, tag="spsb", name=f"spsb{t}")
                for m2 in range(KH):
                    ps = psum.tile([P, NT], f32, tag="ps", name=f"sps{t}_{m2}")
                    for k2 in range(KSH):
                        nc.tensor.matmul(ps, wsd_sb[:, k2, m2 * P:(m2 + 1) * P],
                                         hs[:, k2, :],
                                         start=(k2 == 0), stop=(k2 == KSH - 1))
                    nc.vector.tensor_copy(out=sp_sb[:, m2, :], in_=ps)
                nc.gpsimd.dma_start(out=sp.ap()[:, t], in_=sp_sb)

            def routed_down_group(g2):
                """Routed expert down-proj for one 512-wide H group -> ro_sb."""
                wdBs = []
                for half in range(2):
                    wdB = wdpool.tile([P, KH, 512], bf16, tag="wd",
                                      name=f"wdB{g2}_{half}")
                    nc.scalar.dma_start(
                        out=wdB,
                        in_=wd3.ap()[:, g2, half * KH:(half + 1) * KH, :])
                    wdBs.append(wdB)
                dps, mmap = [], []
                for ri, q in enumerate(runs):
                    dps.append(psum.tile([P, q, C], f32, tag="ps",
                                         name=f"dps{g2}_{ri}"))
                    for j in range(q):
                        mmap.append((ri, j))
                for k2 in range(MI):
                    wb = wdBs[k2 // KH]
                    kk = k2 % KH
                    for mi in range(4):
                        ri, j = mmap[mi]
                        st = dict(start=(k2 == 0 and j == 0),
                                  stop=(k2 == MI - 1))
                        nc.tensor.matmul(dps[ri][:, j, :],
                                         wb[:, kk, mi * P:(mi + 1) * P],
                                         hT[:, k2, :], **st)
                off = 0
                for ri, q in enumerate(runs):
                    nc.vector.tensor_copy(out=ro_sb[:, g2 * 4 + off:
                                                    g2 * 4 + off + q, :],
                                          in_=dps[ri])
                    off += q

            # Interleave: routed groups are DMA-heavy (4 MB / 7.7 us tensor),
            # shared units are tensor-heavy (resident weights). Spreading the
            # shared work between routed groups keeps both the DMA rings and
            # the PE busy end-to-end; shared_down(3) fills the down-phase
            # DMA-starvation gap.
            routed_gu_group(0)
            shared_gu(0)
            load_x(2)
            routed_gu_group(1)
            shared_down(0)
            routed_gu_group(2)
            shared_gu(1)
            load_x(3)
            routed_gu_group(3)
            shared_down(1)
            routed_gu_group(4)
            shared_gu(2)
            routed_gu_group(5)
            shared_down(2)
            routed_gu_group(6)
            shared_gu(3)
            routed_gu_group(7)
            routed_down_group(0)
            shared_down(3)
            routed_down_group(1)
            routed_down_group(2)
            routed_down_group(3)
            nc.gpsimd.dma_start(out=ro.ap(), in_=ro_sb)

    # Split surplus semaphore waits onto InstEventSemaphore carriers (walrus
    # has a 1-wait limit per instruction).
    import bass_rust
    bass_rust.generate_event_semaphores(nc)
    return nc


def _get_bass(C):
    if C not in _BASS_CACHE:
        _BASS_CACHE[C] = _build_bass(C)
    return _BASS_CACHE[C]


def kernel(**inputs):
    global LAST_RESULT, LAST_NC
    x = np.ascontiguousarray(np.asarray(inputs["x"], dtype=np.float32))
    w_router = np.asarray(inputs["w_router"], dtype=np.float32)
    ws_gate = np.asarray(inputs["ws_gate"], dtype=np.float32)
    ws_up = np.asarray(inputs["ws_up"], dtype=np.float32)
    ws_down = np.asarray(inputs["ws_down"], dtype=np.float32)
    we_gate = np.asarray(inputs["we_gate"], dtype=np.float32)
    we_up = np.asarray(inputs["we_up"], dtype=np.float32)
    we_down = np.asarray(inputs["we_down"], dtype=np.float32)

    # --- top-1 routing on host (tiny) ---
    logits = x @ w_router                      # [T, E]
    top = np.argmax(logits, axis=1)            # [T]
    tv = logits[np.arange(T), top]
    score = (1.0 / (1.0 + np.exp(-tv))).astype(np.float32)
    idxs = [np.nonzero(top == e)[0] for e in range(E)]
    maxn = max(len(i) for i in idxs)
    C = max(P, ((maxn + 15) // 16) * 16)

    nc = _get_bass(C)
    LAST_NC = nc

    # x3[p, t, k, j] = x[t*NT + j, k*128 + p]
    x3 = x.reshape(NCH, NT, KH, P).transpose(3, 0, 2, 1).astype(BF)

    in_maps = []
    for e in range(E):
        idx = idxs[e]
        xe = np.zeros((C, H), np.float32)
        if len(idx):
            xe[:len(idx)] = x[idx] * score[idx, None]
        # xe3[p, k, c] = xe[c, k*128 + p]
        xe3 = xe.reshape(C, KH, P).transpose(2, 1, 0).astype(BF)

        # wgu[p, g, w, k, j] = we_{gate,up}[e][k*128 + p, g*512 + j]
        wgu = np.empty((P, 8, 2, KH, 512), BF)
        wgu[:, :, 0] = we_gate[e].reshape(KH, P, 8, 512).transpose(1, 2, 0, 3)
        wgu[:, :, 1] = we_up[e].reshape(KH, P, 8, 512).transpose(1, 2, 0, 3)
        # wd3[p, g2, k2, j] = we_down[e][k2*128 + p, g2*512 + j]
        wd3 = we_down[e].reshape(MI, P, 4, 512).transpose(1, 2, 0, 3).astype(BF)

        # shared-expert shard for this core
        wsg3 = ws_gate[:, e * ISH:(e + 1) * ISH].reshape(KH, P, ISH) \
            .transpose(1, 0, 2).astype(BF)
        wsu3 = ws_up[:, e * ISH:(e + 1) * ISH].reshape(KH, P, ISH) \
            .transpose(1, 0, 2).astype(BF)
        wsd3 = ws_down[e * ISH:(e + 1) * ISH].reshape(KSH, P, H) \
            .transpose(1, 0, 2).astype(BF)

        in_maps.append({
            "xe3": xe3, "wgu": wgu, "wd3": wd3, "x3": x3,
            "wsg3": wsg3, "wsu3": wsu3, "wsd3": wsd3,
        })

    from concourse.bass_utils import run_bass_kernel_spmd
    res = run_bass_kernel_spmd(nc, in_maps, core_ids=list(range(E)))
    LAST_RESULT = res
    outs = res.results

    # shared partials: sp[p, t, m2, j] -> [token, h], summed over cores
    spsum = np.zeros((P, NCH, KH, NT), np.float32)
    for e in range(E):
        spsum += outs[e]["sp"].astype(np.float32)
    out = np.ascontiguousarray(
        spsum.transpose(1, 3, 2, 0).reshape(T, H))

    # routed: ro[p, m, c] -> [c, h], scatter back by token index
    for e in range(E):
        idx = idxs[e]
        if len(idx):
            roe = outs[e]["ro"].astype(np.float32)
            out[idx] += roe.transpose(2, 1, 0).reshape(C, H)[:len(idx)]
    return out


# revision 13
# speedup vs baseline: 2.0711x; 1.0037x over previous
"""Llama4-style MoE (T=1024, H=2048, I=4096, E=8, top-1) on 8 trn2 NeuronCores.

Sharding: expert-parallel. Core e owns expert e's weights plus a 1/8 I-shard
of the shared expert. Host computes top-1 routing (tiny [1024,8] matmul) and
dispatches each expert's tokens (scaled by the sigmoid router score, padded to
capacity C) to its core. Each core returns its expert's MLP output plus a
partial shared-expert output; host sums the partials and scatters the routed
rows back.

All device-side data is bf16 (weights are streamed once, so HBM traffic is the
roofline: ~63 MB/core ≈ 176 us at 358 GB/s; tensor time at C=144 is ~174 us —
balanced). Host pre-packs every tensor into [128 partitions, ...] layout with
contiguous per-partition lines so each weight DMA is a single 1-4 MB transfer.
Activations live in "transposed" space ([feature, token], feature on
partitions) so no on-chip transposes are needed. PSUM accumulates fp32.

Program order puts the shared expert first: its DMA footprint is small
(~11 MB) but its tensor work is large (~82 us), so the big routed weight
stream (~48 MB) flows underneath it.
"""

import numpy as np
import ml_dtypes

T, H, I, E = 1024, 2048, 4096, 8
P = 128
ISH = I // E          # 512  shared-expert I-shard per core
KH = H // P           # 16
MI = I // P           # 32
KSH = ISH // P        # 4
NT = 256              # shared-expert token chunk
NCH = T // NT         # 4 chunks

BF = ml_dtypes.bfloat16

_BASS_CACHE = {}
LAST_RESULT = None    # BassKernelResults of the most recent run (for test harness)
LAST_NC = None


def _pack_runs(C):
    """Split the 4 m-tiles of a 512-wide group into runs that each fit one
    2 KB PSUM bank ([P, q, C] fp32 with q*C <= 512)."""
    pack = max(1, 512 // C)
    runs = []
    left = 4
    while left:
        q = min(pack, left)
        runs.append(q)
        left -= q
    return runs


def _build_bass(C):
    import concourse.bass as bass
    import concourse.mybir as mybir
    import concourse.tile as tile

    assert C <= 512, f"routed capacity {C} > 512 unsupported"

    f32 = mybir.dt.float32
    bf16 = mybir.dt.bfloat16
    SILU = mybir.ActivationFunctionType.Silu
    MULT = mybir.AluOpType.mult

    nc = bass.Bass(trn_type="TRN2", name=f"moe_bf16_c{C}")

    # --- DRAM tensors, all host-packed to [128, ...] partition-major bf16 ---
    xe3 = nc.dram_tensor("xe3", [P, KH, C], bf16, kind="ExternalInput")
    wgu = nc.dram_tensor("wgu", [P, 8, 2, KH, 512], bf16, kind="ExternalInput")
    wd3 = nc.dram_tensor("wd3", [P, 4, MI, 512], bf16, kind="ExternalInput")
    x3 = nc.dram_tensor("x3", [P, NCH, KH, NT], bf16, kind="ExternalInput")
    wsg3 = nc.dram_tensor("wsg3", [P, KH, ISH], bf16, kind="ExternalInput")
    wsu3 = nc.dram_tensor("wsu3", [P, KH, ISH], bf16, kind="ExternalInput")
    wsd3 = nc.dram_tensor("wsd3", [P, KSH, H], bf16, kind="ExternalInput")
    ro = nc.dram_tensor("ro", [P, KH, C], bf16, kind="ExternalOutput")
    sp = nc.dram_tensor("sp", [P, NCH, KH, NT], bf16, kind="ExternalOutput")

    runs = _pack_runs(C)

    with tile.TileContext(nc) as tc:
        from contextlib import ExitStack

        with ExitStack() as ctx:
            const = ctx.enter_context(tc.tile_pool(name="const", bufs=1))
            xpool = ctx.enter_context(tc.tile_pool(name="xpool", bufs=2))
            wpool = ctx.enter_context(tc.tile_pool(name="wpool", bufs=4))
            wdpool = ctx.enter_context(tc.tile_pool(name="wdpool", bufs=2))
            hsp = ctx.enter_context(tc.tile_pool(name="hsp", bufs=2))
            hbuf = ctx.enter_context(tc.tile_pool(name="hbuf", bufs=2))
            outp = ctx.enter_context(tc.tile_pool(name="outp", bufs=2))
            psum = ctx.enter_context(tc.tile_pool(name="psum", bufs=8, space="PSUM"))
            hTp = ctx.enter_context(tc.tile_pool(name="hTp", bufs=1))

            # DMA ring split: gate/up weight stream on the SP HWDGE ring
            # (nc.sync, ~32 MB), input loads + down-proj weight stream on the
            # ACT HWDGE ring (nc.scalar, ~26 MB), outputs on SWDGE
            # (nc.gpsimd, ~4.6 MB) — three independent rings so fixed costs
            # overlap and aggregate BW approaches the HBM limit.

            # --- early input loads, in tensor-consumption order ---
            xeT = const.tile([P, KH, C], bf16)
            nc.scalar.dma_start(out=xeT, in_=xe3.ap())
            xts = [None] * NCH

            def load_x(t):
                xts[t] = xpool.tile([P, KH, NT], bf16, tag="xt", name=f"xt{t}")
                nc.scalar.dma_start(out=xts[t], in_=x3.ap()[:, t])

            load_x(0)
            wsg_sb = const.tile([P, KH, ISH], bf16)
            nc.scalar.dma_start(out=wsg_sb, in_=wsg3.ap())
            wsu_sb = const.tile([P, KH, ISH], bf16)
            nc.scalar.dma_start(out=wsu_sb, in_=wsu3.ap())
            load_x(1)
            wsd_sb = const.tile([P, KSH, H], bf16)
            nc.scalar.dma_start(out=wsd_sb, in_=wsd3.ap())

            hT = hTp.tile([P, MI, C], bf16)
            ro_sb = const.tile([P, KH, C], bf16, name="ro_sb")

            def routed_gu_group(g):
                """Routed expert gate/up for one 512-wide I group -> hT."""
                wgB = wpool.tile([P, KH, 512], bf16, tag="w", name=f"wgB{g}")
                nc.sync.dma_start(out=wgB, in_=wgu.ap()[:, g, 0])
                wuB = wpool.tile([P, KH, 512], bf16, tag="w", name=f"wuB{g}")
                nc.sync.dma_start(out=wuB, in_=wgu.ap()[:, g, 1])
                gps, ups, mmap = [], [], []
                for ri, q in enumerate(runs):
                    gps.append(psum.tile([P, q, C], f32, tag="ps",
                                         name=f"gps{g}_{ri}"))
                    ups.append(psum.tile([P, q, C], f32, tag="ps",
                                         name=f"ups{g}_{ri}"))
                    for j in range(q):
                        mmap.append((ri, j))
                # gate over all k first, then up: wuB may still be in flight
                # while the gate matmuls run.
                for ps_tiles, wB in ((gps, wgB), (ups, wuB)):
                    for k in range(KH):
                        for mi in range(4):
                            ri, j = mmap[mi]
                            # start only on the first write to each PSUM bank:
                            # start=True clears the whole bank's has_written.
                            st = dict(start=(k == 0 and j == 0),
                                      stop=(k == KH - 1))
                            nc.tensor.matmul(ps_tiles[ri][:, j, :],
                                             wB[:, k, mi * P:(mi + 1) * P],
                                             xeT[:, k, :], **st)
                off = 0
                for ri, q in enumerate(runs):
                    h_sb = hbuf.tile([P, q, C], bf16, tag="hrb",
                                     name=f"hrb{g}_{ri}")
                    nc.scalar.activation(out=h_sb, in_=gps[ri], func=SILU)
                    nc.vector.tensor_tensor(hT[:, g * 4 + off:g * 4 + off + q, :],
                                            h_sb, ups[ri], MULT)
                    off += q

            hss = [None] * NCH

            def shared_gu(t):
                """Shared expert gate/up for one 256-token chunk -> hs[t]."""
                xt = xts[t]
                # pack two [P, NT] fp32 accumulators per PSUM bank
                sg = [psum.tile([P, 2, NT], f32, tag="ps", name=f"sg{t}_{r}")
                      for r in range(2)]
                su = [psum.tile([P, 2, NT], f32, tag="ps", name=f"su{t}_{r}")
                      for r in range(2)]
                for ps_tiles, wB in ((sg, wsg_sb), (su, wsu_sb)):
                    for k in range(KH):
                        for m in range(KSH):
                            st = dict(start=(k == 0 and m % 2 == 0),
                                      stop=(k == KH - 1))
                            nc.tensor.matmul(ps_tiles[m // 2][:, m % 2, :],
                                             wB[:, k, m * P:(m + 1) * P],
                                             xt[:, k, :], **st)
                hs = hsp.tile([P, KSH, NT], bf16, tag="hs", name=f"hs{t}")
                hss[t] = hs
                for r in range(2):
                    htmp = hbuf.tile([P, 2, NT], bf16, tag="hsb",
                                     name=f"htmp{t}_{r}")
                    nc.scalar.activation(out=htmp, in_=sg[r], func=SILU)
                    nc.vector.tensor_tensor(hs[:, 2 * r:2 * r + 2, :], htmp,
                                            su[r], MULT)

            def shared_down(t):
                """Shared expert down-proj for chunk t -> sp."""
                hs = hss[t]
                sp_sb = outp.tile([P, KH, NT], bf16, tag="spsb", name=f"spsb{t}",
                                  bufs=1)
                for m2 in range(KH):
                    ps = psum.tile([P, NT], f32, tag="ps", name=f"sps{t}_{m2}")
                    for k2 in range(KSH):
                        nc.tensor.matmul(ps, wsd_sb[:, k2, m2 * P:(m2 + 1) * P],
                                         hs[:, k2, :],
                                         start=(k2 == 0), stop=(k2 == KSH - 1))
                    nc.vector.tensor_copy(out=sp_sb[:, m2, :], in_=ps)
                nc.gpsimd.dma_start(out=sp.ap()[:, t], in_=sp_sb)

            def routed_down_group(g2):
                """Routed expert down-proj for one 512-wide H group -> ro_sb."""
                wdBs = []
                for half in range(2):
                    wdB = wdpool.tile([P, KH, 512], bf16, tag="wd",
                                      name=f"wdB{g2}_{half}")
                    # alternate the two HWDGE rings: by the down phase the
                    # sync ring has finished the gate/up stream, so both
                    # rings fetch wd chunks concurrently.
                    eng = nc.scalar if (g2 * 2 + half) % 2 == 0 else nc.sync
                    eng.dma_start(
                        out=wdB,
                        in_=wd3.ap()[:, g2, half * KH:(half + 1) * KH, :])
                    wdBs.append(wdB)
                dps, mmap = [], []
                for ri, q in enumerate(runs):
                    dps.append(psum.tile([P, q, C], f32, tag="ps",
                                         name=f"dps{g2}_{ri}"))
                    for j in range(q):
                        mmap.append((ri, j))
                for k2 in range(MI):
                    wb = wdBs[k2 // KH]
                    kk = k2 % KH
                    for mi in range(4):
                        ri, j = mmap[mi]
                        st = dict(start=(k2 == 0 and j == 0),
                                  stop=(k2 == MI - 1))
                        nc.tensor.matmul(dps[ri][:, j, :],
                                         wb[:, kk, mi * P:(mi + 1) * P],
                                         hT[:, k2, :], **st)
                off = 0
                for ri, q in enumerate(runs):
                    nc.vector.tensor_copy(out=ro_sb[:, g2 * 4 + off:
                                                    g2 * 4 + off + q, :],
                                          in_=dps[ri])
                    off += q

            # Interleave: routed groups are DMA-heavy (4 MB / 7.7 us tensor),
            # shared units are tensor-heavy (resident weights). Spreading the
            # shared work between routed groups keeps both the DMA rings and
            # the PE busy end-to-end; shared_down(3) fills the down-phase
            # DMA-starvation gap.
            routed_gu_group(0)
            shared_gu(0)
            load_x(2)
            routed_gu_group(1)
            shared_down(0)
            routed_gu_group(2)
            shared_gu(1)
            routed_gu_group(3)
            shared_down(1)
            routed_gu_group(4)
            shared_gu(2)
            routed_gu_group(5)
            shared_down(2)
            routed_gu_group(6)
            load_x(3)
            routed_gu_group(7)
            # down phase is DMA-heavy (16 MB wd vs 31 us tensor): fill the
            # starvation gaps with the last shared chunk.
            routed_down_group(0)
            shared_gu(3)
            routed_down_group(1)
            shared_down(3)
            routed_down_group(2)
            routed_down_group(3)
            nc.gpsimd.dma_start(out=ro.ap(), in_=ro_sb)

    # Split surplus semaphore waits onto InstEventSemaphore carriers (walrus
    # has a 1-wait limit per instruction).
    import bass_rust
    bass_rust.generate_event_semaphores(nc)
    return nc


def _get_bass(C):
    if C not in _BASS_CACHE:
        _BASS_CACHE[C] = _build_bass(C)
    return _BASS_CACHE[C]


def kernel(**inputs):
    global LAST_RESULT, LAST_NC
    x = np.ascontiguousarray(np.asarray(inputs["x"], dtype=np.float32))
    w_router = np.asarray(inputs["w_router"], dtype=np.float32)
    ws_gate = np.asarray(inputs["ws_gate"], dtype=np.float32)
    ws_up = np.asarray(inputs["ws_up"], dtype=np.float32)
    ws_down = np.asarray(inputs["ws_down"], dtype=np.float32)
    we_gate = np.asarray(inputs["we_gate"], dtype=np.float32)
    we_up = np.asarray(inputs["we_up"], dtype=np.float32)
    we_down = np.asarray(inputs["we_down"], dtype=np.float32)

    # --- top-1 routing on host (tiny) ---
    logits = x @ w_router                      # [T, E]
    top = np.argmax(logits, axis=1)            # [T]
    tv = logits[np.arange(T), top]
    score = (1.0 / (1.0 + np.exp(-tv))).astype(np.float32)
    idxs = [np.nonzero(top == e)[0] for e in range(E)]
    maxn = max(len(i) for i in idxs)
    C = max(P, ((maxn + 15) // 16) * 16)

    nc = _get_bass(C)
    LAST_NC = nc

    # x3[p, t, k, j] = x[t*NT + j, k*128 + p]
    x3 = x.reshape(NCH, NT, KH, P).transpose(3, 0, 2, 1).astype(BF)

    in_maps = []
    for e in range(E):
        idx = idxs[e]
        xe = np.zeros((C, H), np.float32)
        if len(idx):
            xe[:len(idx)] = x[idx] * score[idx, None]
        # xe3[p, k, c] = xe[c, k*128 + p]
        xe3 = xe.reshape(C, KH, P).transpose(2, 1, 0).astype(BF)

        # wgu[p, g, w, k, j] = we_{gate,up}[e][k*128 + p, g*512 + j]
        wgu = np.empty((P, 8, 2, KH, 512), BF)
        wgu[:, :, 0] = we_gate[e].reshape(KH, P, 8, 512).transpose(1, 2, 0, 3)
        wgu[:, :, 1] = we_up[e].reshape(KH, P, 8, 512).transpose(1, 2, 0, 3)
        # wd3[p, g2, k2, j] = we_down[e][k2*128 + p, g2*512 + j]
        wd3 = we_down[e].reshape(MI, P, 4, 512).transpose(1, 2, 0, 3).astype(BF)

        # shared-expert shard for this core
        wsg3 = ws_gate[:, e * ISH:(e + 1) * ISH].reshape(KH, P, ISH) \
            .transpose(1, 0, 2).astype(BF)
        wsu3 = ws_up[:, e * ISH:(e + 1) * ISH].reshape(KH, P, ISH) \
            .transpose(1, 0, 2).astype(BF)
        wsd3 = ws_down[e * ISH:(e + 1) * ISH].reshape(KSH, P, H) \
            .transpose(1, 0, 2).astype(BF)

        in_maps.append({
            "xe3": xe3, "wgu": wgu, "wd3": wd3, "x3": x3,
            "wsg3": wsg3, "wsu3": wsu3, "wsd3": wsd3,
        })

    from concourse.bass_utils import run_bass_kernel_spmd
    res = run_bass_kernel_spmd(nc, in_maps, core_ids=list(range(E)))
    LAST_RESULT = res
    outs = res.results

    # shared partials: sp[p, t, m2, j] -> [token, h], summed over cores
    spsum = np.zeros((P, NCH, KH, NT), np.float32)
    for e in range(E):
        spsum += outs[e]["sp"].astype(np.float32)
    out = np.ascontiguousarray(
        spsum.transpose(1, 3, 2, 0).reshape(T, H))

    # routed: ro[p, m, c] -> [c, h], scatter back by token index
    for e in range(E):
        idx = idxs[e]
        if len(idx):
            roe = outs[e]["ro"].astype(np.float32)
            out[idx] += roe.transpose(2, 1, 0).reshape(C, H)[:len(idx)]
    return out


# revision 17
# speedup vs baseline: 2.0869x; 1.0076x over previous
"""Llama4-style MoE (T=1024, H=2048, I=4096, E=8, top-1) on 8 trn2 NeuronCores.

Sharding: expert-parallel. Core e owns expert e's weights plus a 1/8 I-shard
of the shared expert. Host computes top-1 routing (tiny [1024,8] matmul) and
dispatches each expert's tokens (scaled by the sigmoid router score, padded to
capacity C) to its core. Each core returns its expert's MLP output plus a
partial shared-expert output; host sums the partials and scatters the routed
rows back.

All device-side data is bf16 (weights are streamed once, so HBM traffic is the
roofline: ~63 MB/core ≈ 176 us at 358 GB/s; tensor time at C=144 is ~174 us —
balanced). Host pre-packs every tensor into [128 partitions, ...] layout with
contiguous per-partition lines so each weight DMA is a single 1-4 MB transfer.
Activations live in "transposed" space ([feature, token], feature on
partitions) so no on-chip transposes are needed. PSUM accumulates fp32.

Program order puts the shared expert first: its DMA footprint is small
(~11 MB) but its tensor work is large (~82 us), so the big routed weight
stream (~48 MB) flows underneath it.
"""

import numpy as np
import ml_dtypes

T, H, I, E = 1024, 2048, 4096, 8
P = 128
ISH = I // E          # 512  shared-expert I-shard per core
KH = H // P           # 16
MI = I // P           # 32
KSH = ISH // P        # 4
NT = 256              # shared-expert token chunk
NCH = T // NT         # 4 chunks

BF = ml_dtypes.bfloat16

_BASS_CACHE = {}
LAST_RESULT = None    # BassKernelResults of the most recent run (for test harness)
LAST_NC = None


def _pack_runs(C):
    """Split the 4 m-tiles of a 512-wide group into runs that each fit one
    2 KB PSUM bank ([P, q, C] fp32 with q*C <= 512)."""
    pack = max(1, 512 // C)
    runs = []
    left = 4
    while left:
        q = min(pack, left)
        runs.append(q)
        left -= q
    return runs


def _build_bass(C):
    import concourse.bass as bass
    import concourse.mybir as mybir
    import concourse.tile as tile

    assert C <= 512, f"routed capacity {C} > 512 unsupported"

    f32 = mybir.dt.float32
    bf16 = mybir.dt.bfloat16
    SILU = mybir.ActivationFunctionType.Silu
    MULT = mybir.AluOpType.mult

    nc = bass.Bass(trn_type="TRN2", name=f"moe_bf16_c{C}")

    # --- DRAM tensors, all host-packed to [128, ...] partition-major bf16 ---
    xe3 = nc.dram_tensor("xe3", [P, KH, C], bf16, kind="ExternalInput")
    wgu = nc.dram_tensor("wgu", [P, 8, 2, KH, 512], bf16, kind="ExternalInput")
    wd3 = nc.dram_tensor("wd3", [P, 4, MI, 512], bf16, kind="ExternalInput")
    x3 = nc.dram_tensor("x3", [P, NCH, KH, NT], bf16, kind="ExternalInput")
    wsg3 = nc.dram_tensor("wsg3", [P, KH, ISH], bf16, kind="ExternalInput")
    wsu3 = nc.dram_tensor("wsu3", [P, KH, ISH], bf16, kind="ExternalInput")
    wsd3 = nc.dram_tensor("wsd3", [P, KSH, H], bf16, kind="ExternalInput")
    ro = nc.dram_tensor("ro", [P, KH, C], bf16, kind="ExternalOutput")
    sp = nc.dram_tensor("sp", [P, NCH, KH, NT], bf16, kind="ExternalOutput")

    runs = _pack_runs(C)

    with tile.TileContext(nc) as tc:
        from contextlib import ExitStack

        with ExitStack() as ctx:
            const = ctx.enter_context(tc.tile_pool(name="const", bufs=1))
            xpool = ctx.enter_context(tc.tile_pool(name="xpool", bufs=2))
            wpool = ctx.enter_context(tc.tile_pool(name="wpool", bufs=8))
            wdpool = ctx.enter_context(tc.tile_pool(name="wdpool", bufs=4))
            hsp = ctx.enter_context(tc.tile_pool(name="hsp", bufs=2))
            hbuf = ctx.enter_context(tc.tile_pool(name="hbuf", bufs=2))
            outp = ctx.enter_context(tc.tile_pool(name="outp", bufs=2))
            psum = ctx.enter_context(tc.tile_pool(name="psum", bufs=8, space="PSUM"))
            hTp = ctx.enter_context(tc.tile_pool(name="hTp", bufs=1))

            # DMA ring split: gate/up weight stream on the SP HWDGE ring
            # (nc.sync, ~32 MB), input loads + down-proj weight stream on the
            # ACT HWDGE ring (nc.scalar, ~26 MB), outputs on SWDGE
            # (nc.gpsimd, ~4.6 MB) — three independent rings so fixed costs
            # overlap and aggregate BW approaches the HBM limit.

            # --- early input loads, in tensor-consumption order ---
            xeT = const.tile([P, KH, C], bf16)
            nc.scalar.dma_start(out=xeT, in_=xe3.ap())
            xts = [None] * NCH

            def load_x(t):
                xts[t] = xpool.tile([P, KH, NT], bf16, tag="xt", name=f"xt{t}")
                nc.scalar.dma_start(out=xts[t], in_=x3.ap()[:, t])

            load_x(0)
            wsg_sb = const.tile([P, KH, ISH], bf16)
            nc.scalar.dma_start(out=wsg_sb, in_=wsg3.ap())
            wsu_sb = const.tile([P, KH, ISH], bf16)
            nc.scalar.dma_start(out=wsu_sb, in_=wsu3.ap())
            load_x(1)
            wsd_sb = const.tile([P, KSH, H], bf16)
            nc.scalar.dma_start(out=wsd_sb, in_=wsd3.ap())

            hT = hTp.tile([P, MI, C], bf16)
            ro_sb = const.tile([P, KH, C], bf16, name="ro_sb")

            KHH = KH // 2  # half-group k tiles (1 MB DMA granularity)

            def routed_gu_group(g):
                """Routed expert gate/up for one 512-wide I group -> hT.

                Weights stream in four 1 MB half-tiles (gate k0-7, gate k8-15,
                up k0-7, up k8-15) in consumption order, so the matmuls start
                as soon as the first megabyte lands."""
                halves = []
                for w in range(2):
                    for hh in range(2):
                        wB = wpool.tile([P, KHH, 512], bf16, tag="w",
                                        name=f"w{g}_{w}_{hh}")
                        nc.sync.dma_start(
                            out=wB,
                            in_=wgu.ap()[:, g, w, hh * KHH:(hh + 1) * KHH, :])
                        halves.append(wB)
                gps, ups, mmap = [], [], []
                for ri, q in enumerate(runs):
                    gps.append(psum.tile([P, q, C], f32, tag="ps",
                                         name=f"gps{g}_{ri}"))
                    ups.append(psum.tile([P, q, C], f32, tag="ps",
                                         name=f"ups{g}_{ri}"))
                    for j in range(q):
                        mmap.append((ri, j))
                for w, ps_tiles in ((0, gps), (1, ups)):
                    for k in range(KH):
                        wB = halves[w * 2 + k // KHH]
                        for mi in range(4):
                            ri, j = mmap[mi]
                            # start only on the first write to each PSUM bank:
                            # start=True clears the whole bank's has_written.
                            st = dict(start=(k == 0 and j == 0),
                                      stop=(k == KH - 1))
                            nc.tensor.matmul(ps_tiles[ri][:, j, :],
                                             wB[:, k % KHH, mi * P:(mi + 1) * P],
                                             xeT[:, k, :], **st)
                off = 0
                for ri, q in enumerate(runs):
                    h_sb = hbuf.tile([P, q, C], bf16, tag="hrb",
                                     name=f"hrb{g}_{ri}")
                    nc.scalar.activation(out=h_sb, in_=gps[ri], func=SILU)
                    nc.vector.tensor_tensor(hT[:, g * 4 + off:g * 4 + off + q, :],
                                            h_sb, ups[ri], MULT)
                    off += q

            hss = [None] * NCH

            def shared_gu(t):
                """Shared expert gate/up for one 256-token chunk -> hs[t]."""
                xt = xts[t]
                # pack two [P, NT] fp32 accumulators per PSUM bank
                sg = [psum.tile([P, 2, NT], f32, tag="ps", name=f"sg{t}_{r}")
                      for r in range(2)]
                su = [psum.tile([P, 2, NT], f32, tag="ps", name=f"su{t}_{r}")
                      for r in range(2)]
                for ps_tiles, wB in ((sg, wsg_sb), (su, wsu_sb)):
                    for k in range(KH):
                        for m in range(KSH):
                            st = dict(start=(k == 0 and m % 2 == 0),
                                      stop=(k == KH - 1))
                            nc.tensor.matmul(ps_tiles[m // 2][:, m % 2, :],
                                             wB[:, k, m * P:(m + 1) * P],
                                             xt[:, k, :], **st)
                hs = hsp.tile([P, KSH, NT], bf16, tag="hs", name=f"hs{t}")
                hss[t] = hs
                for r in range(2):
                    htmp = hbuf.tile([P, 2, NT], bf16, tag="hsb",
                                     name=f"htmp{t}_{r}")
                    nc.scalar.activation(out=htmp, in_=sg[r], func=SILU)
                    nc.vector.tensor_tensor(hs[:, 2 * r:2 * r + 2, :], htmp,
                                            su[r], MULT)

            def shared_down(t):
                """Shared expert down-proj for chunk t -> sp."""
                hs = hss[t]
                sp_sb = outp.tile([P, KH, NT], bf16, tag="spsb", name=f"spsb{t}",
                                  bufs=1)
                for m2 in range(KH):
                    ps = psum.tile([P, NT], f32, tag="ps", name=f"sps{t}_{m2}")
                    for k2 in range(KSH):
                        nc.tensor.matmul(ps, wsd_sb[:, k2, m2 * P:(m2 + 1) * P],
                                         hs[:, k2, :],
                                         start=(k2 == 0), stop=(k2 == KSH - 1))
                    nc.vector.tensor_copy(out=sp_sb[:, m2, :], in_=ps)
                nc.gpsimd.dma_start(out=sp.ap()[:, t], in_=sp_sb)

            # wd streams as 16 x 1 MB chunks (8 k2-tiles each) through a
            # 4-deep rolling prefetch: chunks for down-group g2+1 are
            # triggered at the start of group g2, and the first four are
            # hoisted into the gate/up phase (see program order below).
            wd_tiles = {}

            def prefetch_wd(c):
                if c in wd_tiles or c >= 16:
                    return
                wdB = wdpool.tile([P, KHH, 512], bf16, tag="wd",
                                  name=f"wdB{c}")
                # alternate the two HWDGE rings: by the down phase the sync
                # ring has finished the gate/up stream, so both rings fetch
                # wd chunks concurrently.
                eng = nc.scalar if c % 2 == 0 else nc.sync
                g2, q = c // 4, c % 4
                eng.dma_start(
                    out=wdB, in_=wd3.ap()[:, g2, q * KHH:(q + 1) * KHH, :])
                wd_tiles[c] = wdB

            def routed_down_group(g2):
                """Routed expert down-proj for one 512-wide H group -> ro_sb."""
                for i in range(4):
                    prefetch_wd((g2 + 1) * 4 + i)
                dps, mmap = [], []
                for ri, q in enumerate(runs):
                    dps.append(psum.tile([P, q, C], f32, tag="ps",
                                         name=f"dps{g2}_{ri}"))
                    for j in range(q):
                        mmap.append((ri, j))
                for k2 in range(MI):
                    wb = wd_tiles[g2 * 4 + k2 // KHH]
                    kk = k2 % KHH
                    for mi in range(4):
                        ri, j = mmap[mi]
                        st = dict(start=(k2 == 0 and j == 0),
                                  stop=(k2 == MI - 1))
                        nc.tensor.matmul(dps[ri][:, j, :],
                                         wb[:, kk, mi * P:(mi + 1) * P],
                                         hT[:, k2, :], **st)
                off = 0
                for ri, q in enumerate(runs):
                    nc.vector.tensor_copy(out=ro_sb[:, g2 * 4 + off:
                                                    g2 * 4 + off + q, :],
                                          in_=dps[ri])
                    off += q

            # Interleave: routed groups are DMA-heavy (4 MB / 7.7 us tensor),
            # shared units are tensor-heavy (resident weights). Spreading the
            # shared work between routed groups keeps both the DMA rings and
            # the PE busy end-to-end; shared_down(3) fills the down-phase
            # DMA-starvation gap.
            routed_gu_group(0)
            shared_gu(0)
            load_x(2)
            routed_gu_group(1)
            shared_down(0)
            routed_gu_group(2)
            shared_gu(1)
            routed_gu_group(3)
            shared_down(1)
            routed_gu_group(4)
            shared_gu(2)
            routed_gu_group(5)
            shared_down(2)
            routed_gu_group(6)
            load_x(3)
            for c in range(4):
                prefetch_wd(c)
            routed_gu_group(7)
            # down phase is DMA-heavy (16 MB wd vs 31 us tensor): fill the
            # starvation gaps with the last shared chunk.
            routed_down_group(0)
            shared_gu(3)
            routed_down_group(1)
            shared_down(3)
            routed_down_group(2)
            routed_down_group(3)
            nc.gpsimd.dma_start(out=ro.ap(), in_=ro_sb)

    # Split surplus semaphore waits onto InstEventSemaphore carriers (walrus
    # has a 1-wait limit per instruction).
    import bass_rust
    bass_rust.generate_event_semaphores(nc)
    return nc


def _get_bass(C):
    if C not in _BASS_CACHE:
        _BASS_CACHE[C] = _build_bass(C)
    return _BASS_CACHE[C]


def kernel(**inputs):
    global LAST_RESULT, LAST_NC
    x = np.ascontiguousarray(np.asarray(inputs["x"], dtype=np.float32))
    w_router = np.asarray(inputs["w_router"], dtype=np.float32)
    ws_gate = np.asarray(inputs["ws_gate"], dtype=np.float32)
    ws_up = np.asarray(inputs["ws_up"], dtype=np.float32)
    ws_down = np.asarray(inputs["ws_down"], dtype=np.float32)
    we_gate = np.asarray(inputs["we_gate"], dtype=np.float32)
    we_up = np.asarray(inputs["we_up"], dtype=np.float32)
    we_down = np.asarray(inputs["we_down"], dtype=np.float32)

    # --- top-1 routing on host (tiny) ---
    logits = x @ w_router                      # [T, E]
    top = np.argmax(logits, axis=1)            # [T]
    tv = logits[np.arange(T), top]
    score = (1.0 / (1.0 + np.exp(-tv))).astype(np.float32)
    idxs = [np.nonzero(top == e)[0] for e in range(E)]
    maxn = max(len(i) for i in idxs)
    C = max(P, ((maxn + 15) // 16) * 16)

    nc = _get_bass(C)
    LAST_NC = nc

    # x3[p, t, k, j] = x[t*NT + j, k*128 + p]
    x3 = x.reshape(NCH, NT, KH, P).transpose(3, 0, 2, 1).astype(BF)

    in_maps = []
    for e in range(E):
        idx = idxs[e]
        xe = np.zeros((C, H), np.float32)
        if len(idx):
            xe[:len(idx)] = x[idx] * score[idx, None]
        # xe3[p, k, c] = xe[c, k*128 + p]
        xe3 = xe.reshape(C, KH, P).transpose(2, 1, 0).astype(BF)

        # wgu[p, g, w, k, j] = we_{gate,up}[e][k*128 + p, g*512 + j]
        wgu = np.empty((P, 8, 2, KH, 512), BF)
        wgu[:, :, 0] = we_gate[e].reshape(KH, P, 8, 512).transpose(1, 2, 0, 3)
        wgu[:, :, 1] = we_up[e].reshape(KH, P, 8, 512).transpose(1, 2, 0, 3)
        # wd3[p, g2, k2, j] = we_down[e][k2*128 + p, g2*512 + j]
        wd3 = we_down[e].reshape(MI, P, 4, 512).transpose(1, 2, 0, 3).astype(BF)

        # shared-expert shard for this core
        wsg3 = ws_gate[:, e * ISH:(e + 1) * ISH].reshape(KH, P, ISH) \
            .transpose(1, 0, 2).astype(BF)
        wsu3 = ws_up[:, e * ISH:(e + 1) * ISH].reshape(KH, P, ISH) \
            .transpose(1, 0, 2).astype(BF)
        wsd3 = ws_down[e * ISH:(e + 1) * ISH].reshape(KSH, P, H) \
            .transpose(1, 0, 2).astype(BF)

        in_maps.append({
            "xe3": xe3, "wgu": wgu, "wd3": wd3, "x3": x3,
            "wsg3": wsg3, "wsu3": wsu3, "wsd3": wsd3,
        })

    from concourse.bass_utils import run_bass_kernel_spmd
    res = run_bass_kernel_spmd(nc, in_maps, core_ids=list(range(E)))
    LAST_RESULT = res
    outs = res.results

    # shared partials: sp[p, t, m2, j] -> [token, h], summed over cores
    spsum = np.zeros((P, NCH, KH, NT), np.float32)
    for e in range(E):
        spsum += outs[e]["sp"].astype(np.float32)
    out = np.ascontiguousarray(
        spsum.transpose(1, 3, 2, 0).reshape(T, H))

    # routed: ro[p, m, c] -> [c, h], scatter back by token index
    for e in range(E):
        idx = idxs[e]
        if len(idx):
            roe = outs[e]["ro"].astype(np.float32)
            out[idx] += roe.transpose(2, 1, 0).reshape(C, H)[:len(idx)]
    return out


# revision 20
# speedup vs baseline: 2.1076x; 1.0099x over previous
"""Llama4-style MoE (T=1024, H=2048, I=4096, E=8, top-1) on 8 trn2 NeuronCores.

Sharding: expert-parallel. Core e owns expert e's weights plus a 1/8 I-shard
of the shared expert. Host computes top-1 routing (tiny [1024,8] matmul) and
dispatches each expert's tokens (scaled by the sigmoid router score, padded to
capacity C) to its core. Each core returns its expert's MLP output plus a
partial shared-expert output; host sums the partials and scatters the routed
rows back.

All device-side data is bf16 (weights are streamed once, so HBM traffic is the
roofline: ~63 MB/core ≈ 176 us at 358 GB/s; tensor time at C=144 is ~174 us —
balanced). Host pre-packs every tensor into [128 partitions, ...] layout with
contiguous per-partition lines so each weight DMA is a single 1-4 MB transfer.
Activations live in "transposed" space ([feature, token], feature on
partitions) so no on-chip transposes are needed. PSUM accumulates fp32.

Program order puts the shared expert first: its DMA footprint is small
(~11 MB) but its tensor work is large (~82 us), so the big routed weight
stream (~48 MB) flows underneath it.
"""

import numpy as np
import ml_dtypes

T, H, I, E = 1024, 2048, 4096, 8
P = 128
ISH = I // E          # 512  shared-expert I-shard per core
KH = H // P           # 16
MI = I // P           # 32
KSH = ISH // P        # 4
NT = 256              # shared-expert token chunk
NCH = T // NT         # 4 chunks

BF = ml_dtypes.bfloat16

_BASS_CACHE = {}
LAST_RESULT = None    # BassKernelResults of the most recent run (for test harness)
LAST_NC = None


def _pack_runs(C):
    """Split the 4 m-tiles of a 512-wide group into runs that each fit one
    2 KB PSUM bank ([P, q, C] fp32 with q*C <= 512)."""
    pack = max(1, 512 // C)
    runs = []
    left = 4
    while left:
        q = min(pack, left)
        runs.append(q)
        left -= q
    return runs


def _build_bass(C):
    import concourse.bass as bass
    import concourse.mybir as mybir
    import concourse.tile as tile

    assert C <= 512, f"routed capacity {C} > 512 unsupported"

    f32 = mybir.dt.float32
    bf16 = mybir.dt.bfloat16
    SILU = mybir.ActivationFunctionType.Silu
    MULT = mybir.AluOpType.mult

    nc = bass.Bass(trn_type="TRN2", name=f"moe_bf16_c{C}")

    # --- DRAM tensors, all host-packed to [128, ...] partition-major bf16 ---
    xe3 = nc.dram_tensor("xe3", [P, KH, C], bf16, kind="ExternalInput")
    wgu = nc.dram_tensor("wgu", [P, 8, 2, KH, 512], bf16, kind="ExternalInput")
    wd3 = nc.dram_tensor("wd3", [P, 4, MI, 512], bf16, kind="ExternalInput")
    x3 = nc.dram_tensor("x3", [P, NCH, KH, NT], bf16, kind="ExternalInput")
    wsg3 = nc.dram_tensor("wsg3", [P, KH, ISH], bf16, kind="ExternalInput")
    wsu3 = nc.dram_tensor("wsu3", [P, KH, ISH], bf16, kind="ExternalInput")
    wsd3 = nc.dram_tensor("wsd3", [P, KSH, H], bf16, kind="ExternalInput")
    ro = nc.dram_tensor("ro", [P, KH, C], bf16, kind="ExternalOutput")
    sp = nc.dram_tensor("sp", [P, NCH, KH, NT], bf16, kind="ExternalOutput")

    runs = _pack_runs(C)

    with tile.TileContext(nc) as tc:
        from contextlib import ExitStack

        with ExitStack() as ctx:
            const = ctx.enter_context(tc.tile_pool(name="const", bufs=1))
            xpool = ctx.enter_context(tc.tile_pool(name="xpool", bufs=2))
            wpool = ctx.enter_context(tc.tile_pool(name="wpool", bufs=8))
            wdpool = ctx.enter_context(tc.tile_pool(name="wdpool", bufs=4))
            hsp = ctx.enter_context(tc.tile_pool(name="hsp", bufs=2))
            hbuf = ctx.enter_context(tc.tile_pool(name="hbuf", bufs=2))
            outp = ctx.enter_context(tc.tile_pool(name="outp", bufs=2))
            psum = ctx.enter_context(tc.tile_pool(name="psum", bufs=8, space="PSUM"))
            hTp = ctx.enter_context(tc.tile_pool(name="hTp", bufs=1))

            # DMA ring split: gate/up weight stream on the SP HWDGE ring
            # (nc.sync, ~32 MB), input loads + down-proj weight stream on the
            # ACT HWDGE ring (nc.scalar, ~26 MB), outputs on SWDGE
            # (nc.gpsimd, ~4.6 MB) — three independent rings so fixed costs
            # overlap and aggregate BW approaches the HBM limit.

            # --- early input loads, in tensor-consumption order ---
            xeT = const.tile([P, KH, C], bf16)
            nc.scalar.dma_start(out=xeT, in_=xe3.ap())
            xts = [None] * NCH

            def load_x(t):
                xts[t] = xpool.tile([P, KH, NT], bf16, tag="xt", name=f"xt{t}")
                nc.scalar.dma_start(out=xts[t], in_=x3.ap()[:, t])

            load_x(0)
            wsg_sb = const.tile([P, KH, ISH], bf16)
            nc.scalar.dma_start(out=wsg_sb, in_=wsg3.ap())
            wsu_sb = const.tile([P, KH, ISH], bf16)
            nc.scalar.dma_start(out=wsu_sb, in_=wsu3.ap())
            load_x(1)
            wsd_sb = const.tile([P, KSH, H], bf16)
            nc.scalar.dma_start(out=wsd_sb, in_=wsd3.ap())

            hT = hTp.tile([P, MI, C], bf16)
            ro_sb = const.tile([P, KH, C], bf16, name="ro_sb")

            # --- PE warm-up: ~30 throwaway matmuls on the routed tokens ---
            # The PE clock boots throttled (K=4/8, 1.2 GHz) and needs ~3.4 us
            # of sustained activity to reach 2.4 GHz. The head of this kernel
            # is DMA-arrival-bound anyway, so spend it warming the clock; the
            # results land in a scratch PSUM slot and are never read. Real
            # accumulations later reclaim the bank via start=True (which
            # clears has_written), so the residue is harmless.
            warm = psum.tile([P, C], f32, tag="ps", name="warm")
            for _ in range(30):
                nc.tensor.matmul(warm, xeT[:, 0, 0:P], xeT[:, 0, :],
                                 start=True, stop=True)

            KHH = KH // 2  # half-group k tiles (1 MB DMA granularity)

            def routed_gu_group(g):
                """Routed expert gate/up for one 512-wide I group -> hT.

                Weights stream in four 1 MB half-tiles (gate k0-7, gate k8-15,
                up k0-7, up k8-15) in consumption order, so the matmuls start
                as soon as the first megabyte lands."""
                halves = []
                for w in range(2):
                    for hh in range(2):
                        wB = wpool.tile([P, KHH, 512], bf16, tag="w",
                                        name=f"w{g}_{w}_{hh}")
                        nc.sync.dma_start(
                            out=wB,
                            in_=wgu.ap()[:, g, w, hh * KHH:(hh + 1) * KHH, :])
                        halves.append(wB)
                gps, ups, mmap = [], [], []
                for ri, q in enumerate(runs):
                    gps.append(psum.tile([P, q, C], f32, tag="ps",
                                         name=f"gps{g}_{ri}"))
                    ups.append(psum.tile([P, q, C], f32, tag="ps",
                                         name=f"ups{g}_{ri}"))
                    for j in range(q):
                        mmap.append((ri, j))
                for w, ps_tiles in ((0, gps), (1, ups)):
                    for k in range(KH):
                        wB = halves[w * 2 + k // KHH]
                        for mi in range(4):
                            ri, j = mmap[mi]
                            # start only on the first write to each PSUM bank:
                            # start=True clears the whole bank's has_written.
                            st = dict(start=(k == 0 and j == 0),
                                      stop=(k == KH - 1))
                            nc.tensor.matmul(ps_tiles[ri][:, j, :],
                                             wB[:, k % KHH, mi * P:(mi + 1) * P],
                                             xeT[:, k, :], **st)
                off = 0
                for ri, q in enumerate(runs):
                    h_sb = hbuf.tile([P, q, C], bf16, tag="hrb",
                                     name=f"hrb{g}_{ri}")
                    nc.scalar.activation(out=h_sb, in_=gps[ri], func=SILU)
                    nc.vector.tensor_tensor(hT[:, g * 4 + off:g * 4 + off + q, :],
                                            h_sb, ups[ri], MULT)
                    off += q

            hss = [None] * NCH

            def shared_gu(t):
                """Shared expert gate/up for one 256-token chunk -> hs[t]."""
                xt = xts[t]
                # pack two [P, NT] fp32 accumulators per PSUM bank
                sg = [psum.tile([P, 2, NT], f32, tag="ps", name=f"sg{t}_{r}")
                      for r in range(2)]
                su = [psum.tile([P, 2, NT], f32, tag="ps", name=f"su{t}_{r}")
                      for r in range(2)]
                for ps_tiles, wB in ((sg, wsg_sb), (su, wsu_sb)):
                    for k in range(KH):
                        for m in range(KSH):
                            st = dict(start=(k == 0 and m % 2 == 0),
                                      stop=(k == KH - 1))
                            nc.tensor.matmul(ps_tiles[m // 2][:, m % 2, :],
                                             wB[:, k, m * P:(m + 1) * P],
                                             xt[:, k, :], **st)
                hs = hsp.tile([P, KSH, NT], bf16, tag="hs", name=f"hs{t}")
                hss[t] = hs
                for r in range(2):
                    htmp = hbuf.tile([P, 2, NT], bf16, tag="hsb",
                                     name=f"htmp{t}_{r}")
                    nc.scalar.activation(out=htmp, in_=sg[r], func=SILU)
                    nc.vector.tensor_tensor(hs[:, 2 * r:2 * r + 2, :], htmp,
                                            su[r], MULT)

            def shared_down(t):
                """Shared expert down-proj for chunk t -> sp."""
                hs = hss[t]
                sp_sb = outp.tile([P, KH, NT], bf16, tag="spsb", name=f"spsb{t}",
                                  bufs=1)
                for m2 in range(KH):
                    ps = psum.tile([P, NT], f32, tag="ps", name=f"sps{t}_{m2}")
                    for k2 in range(KSH):
                        nc.tensor.matmul(ps, wsd_sb[:, k2, m2 * P:(m2 + 1) * P],
                                         hs[:, k2, :],
                                         start=(k2 == 0), stop=(k2 == KSH - 1))
                    nc.vector.tensor_copy(out=sp_sb[:, m2, :], in_=ps)
                nc.gpsimd.dma_start(out=sp.ap()[:, t], in_=sp_sb)

            # wd streams as 16 x 1 MB chunks (8 k2-tiles each) through a
            # 4-deep rolling prefetch: chunks for down-group g2+1 are
            # triggered at the start of group g2, and the first four are
            # hoisted into the gate/up phase (see program order below).
            wd_tiles = {}

            def prefetch_wd(c):
                if c in wd_tiles or c >= 16:
                    return
                wdB = wdpool.tile([P, KHH, 512], bf16, tag="wd",
                                  name=f"wdB{c}")
                # alternate the two HWDGE rings: by the down phase the sync
                # ring has finished the gate/up stream, so both rings fetch
                # wd chunks concurrently.
                eng = nc.scalar if c % 2 == 0 else nc.sync
                g2, q = c // 4, c % 4
                eng.dma_start(
                    out=wdB, in_=wd3.ap()[:, g2, q * KHH:(q + 1) * KHH, :])
                wd_tiles[c] = wdB

            def routed_down_group(g2):
                """Routed expert down-proj for one 512-wide H group -> ro_sb."""
                for i in range(4):
                    prefetch_wd((g2 + 1) * 4 + i)
                dps, mmap = [], []
                for ri, q in enumerate(runs):
                    dps.append(psum.tile([P, q, C], f32, tag="ps",
                                         name=f"dps{g2}_{ri}"))
                    for j in range(q):
                        mmap.append((ri, j))
                for k2 in range(MI):
                    wb = wd_tiles[g2 * 4 + k2 // KHH]
                    kk = k2 % KHH
                    for mi in range(4):
                        ri, j = mmap[mi]
                        st = dict(start=(k2 == 0 and j == 0),
                                  stop=(k2 == MI - 1))
                        nc.tensor.matmul(dps[ri][:, j, :],
                                         wb[:, kk, mi * P:(mi + 1) * P],
                                         hT[:, k2, :], **st)
                off = 0
                for ri, q in enumerate(runs):
                    nc.vector.tensor_copy(out=ro_sb[:, g2 * 4 + off:
                                                    g2 * 4 + off + q, :],
                                          in_=dps[ri])
                    off += q
                nc.gpsimd.dma_start(out=ro.ap()[:, g2 * 4:(g2 + 1) * 4, :],
                                    in_=ro_sb[:, g2 * 4:(g2 + 1) * 4, :])

            # Interleave: routed groups are DMA-heavy (4 MB / 7.7 us tensor),
            # shared units are tensor-heavy (resident weights). Spreading the
            # shared work between routed groups keeps both the DMA rings and
            # the PE busy end-to-end; shared_down(3) fills the down-phase
            # DMA-starvation gap.
            routed_gu_group(0)
            shared_gu(0)
            load_x(2)
            routed_gu_group(1)
            shared_down(0)
            routed_gu_group(2)
            shared_gu(1)
            routed_gu_group(3)
            shared_down(1)
            routed_gu_group(4)
            shared_gu(2)
            load_x(3)
            routed_gu_group(5)
            shared_down(2)
            routed_gu_group(6)
            shared_gu(3)
            for c in range(4):
                prefetch_wd(c)
            routed_gu_group(7)
            # down phase: the rolling wd prefetch keeps both rings streaming;
            # shared_down(3) fills the first starvation window.
            routed_down_group(0)
            shared_down(3)
            routed_down_group(1)
            routed_down_group(2)
            routed_down_group(3)

    # Split surplus semaphore waits onto InstEventSemaphore carriers (walrus
    # has a 1-wait limit per instruction).
    import bass_rust
    bass_rust.generate_event_semaphores(nc)
    return nc


def _get_bass(C):
    if C not in _BASS_CACHE:
        _BASS_CACHE[C] = _build_bass(C)
    return _BASS_CACHE[C]


def kernel(**inputs):
    global LAST_RESULT, LAST_NC
    x = np.ascontiguousarray(np.asarray(inputs["x"], dtype=np.float32))
    w_router = np.asarray(inputs["w_router"], dtype=np.float32)
    ws_gate = np.asarray(inputs["ws_gate"], dtype=np.float32)
    ws_up = np.asarray(inputs["ws_up"], dtype=np.float32)
    ws_down = np.asarray(inputs["ws_down"], dtype=np.float32)
    we_gate = np.asarray(inputs["we_gate"], dtype=np.float32)
    we_up = np.asarray(inputs["we_up"], dtype=np.float32)
    we_down = np.asarray(inputs["we_down"], dtype=np.float32)

    # --- top-1 routing on host (tiny) ---
    logits = x @ w_router                      # [T, E]
    top = np.argmax(logits, axis=1)            # [T]
    tv = logits[np.arange(T), top]
    score = (1.0 / (1.0 + np.exp(-tv))).astype(np.float32)
    idxs = [np.nonzero(top == e)[0] for e in range(E)]
    maxn = max(len(i) for i in idxs)
    C = max(P, ((maxn + 15) // 16) * 16)

    nc = _get_bass(C)
    LAST_NC = nc

    # x3[p, t, k, j] = x[t*NT + j, k*128 + p]
    x3 = x.reshape(NCH, NT, KH, P).transpose(3, 0, 2, 1).astype(BF)

    in_maps = []
    for e in range(E):
        idx = idxs[e]
        xe = np.zeros((C, H), np.float32)
        if len(idx):
            xe[:len(idx)] = x[idx] * score[idx, None]
        # xe3[p, k, c] = xe[c, k*128 + p]
        xe3 = xe.reshape(C, KH, P).transpose(2, 1, 0).astype(BF)

        # wgu[p, g, w, k, j] = we_{gate,up}[e][k*128 + p, g*512 + j]
        wgu = np.empty((P, 8, 2, KH, 512), BF)
        wgu[:, :, 0] = we_gate[e].reshape(KH, P, 8, 512).transpose(1, 2, 0, 3)
        wgu[:, :, 1] = we_up[e].reshape(KH, P, 8, 512).transpose(1, 2, 0, 3)
        # wd3[p, g2, k2, j] = we_down[e][k2*128 + p, g2*512 + j]
        wd3 = we_down[e].reshape(MI, P, 4, 512).transpose(1, 2, 0, 3).astype(BF)

        # shared-expert shard for this core
        wsg3 = ws_gate[:, e * ISH:(e + 1) * ISH].reshape(KH, P, ISH) \
            .transpose(1, 0, 2).astype(BF)
        wsu3 = ws_up[:, e * ISH:(e + 1) * ISH].reshape(KH, P, ISH) \
            .transpose(1, 0, 2).astype(BF)
        wsd3 = ws_down[e * ISH:(e + 1) * ISH].reshape(KSH, P, H) \
            .transpose(1, 0, 2).astype(BF)

        in_maps.append({
            "xe3": xe3, "wgu": wgu, "wd3": wd3, "x3": x3,
            "wsg3": wsg3, "wsu3": wsu3, "wsd3": wsd3,
        })

    from concourse.bass_utils import run_bass_kernel_spmd
    res = run_bass_kernel_spmd(nc, in_maps, core_ids=list(range(E)))
    LAST_RESULT = res
    outs = res.results

    # shared partials: sp[p, t, m2, j] -> [token, h], summed over cores
    spsum = np.zeros((P, NCH, KH, NT), np.float32)
    for e in range(E):
        spsum += outs[e]["sp"].astype(np.float32)
    out = np.ascontiguousarray(
        spsum.transpose(1, 3, 2, 0).reshape(T, H))

    # routed: ro[p, m, c] -> [c, h], scatter back by token index
    for e in range(E):
        idx = idxs[e]
        if len(idx):
            roe = outs[e]["ro"].astype(np.float32)
            out[idx] += roe.transpose(2, 1, 0).reshape(C, H)[:len(idx)]
    return out


# revision 23
# speedup vs baseline: 2.1195x; 1.0057x over previous
"""Llama4-style MoE (T=1024, H=2048, I=4096, E=8, top-1) on 8 trn2 NeuronCores.

Sharding: expert-parallel. Core e owns expert e's weights plus a 1/8 I-shard
of the shared expert. Host computes top-1 routing (tiny [1024,8] matmul) and
dispatches each expert's tokens (scaled by the sigmoid router score, padded to
capacity C) to its core. Each core returns its expert's MLP output plus a
partial shared-expert output; host sums the partials and scatters the routed
rows back.

All device-side data is bf16 (weights are streamed once, so HBM traffic is the
roofline: ~63 MB/core ≈ 176 us at 358 GB/s; tensor time at C=144 is ~174 us —
balanced). Host pre-packs every tensor into [128 partitions, ...] layout with
contiguous per-partition lines so each weight DMA is a single 1-4 MB transfer.
Activations live in "transposed" space ([feature, token], feature on
partitions) so no on-chip transposes are needed. PSUM accumulates fp32.

Program order puts the shared expert first: its DMA footprint is small
(~11 MB) but its tensor work is large (~82 us), so the big routed weight
stream (~48 MB) flows underneath it.
"""

import numpy as np
import ml_dtypes

T, H, I, E = 1024, 2048, 4096, 8
P = 128
ISH = I // E          # 512  shared-expert I-shard per core
KH = H // P           # 16
MI = I // P           # 32
KSH = ISH // P        # 4
NT = 256              # shared-expert token chunk
NCH = T // NT         # 4 chunks

BF = ml_dtypes.bfloat16

_BASS_CACHE = {}
LAST_RESULT = None    # BassKernelResults of the most recent run (for test harness)
LAST_NC = None


def _pack_runs(C):
    """Split the 4 m-tiles of a 512-wide group into runs that each fit one
    2 KB PSUM bank ([P, q, C] fp32 with q*C <= 512)."""
    pack = max(1, 512 // C)
    runs = []
    left = 4
    while left:
        q = min(pack, left)
        runs.append(q)
        left -= q
    return runs


def _build_bass(C):
    import concourse.bass as bass
    import concourse.mybir as mybir
    import concourse.tile as tile

    assert C <= 512, f"routed capacity {C} > 512 unsupported"

    f32 = mybir.dt.float32
    bf16 = mybir.dt.bfloat16
    SILU = mybir.ActivationFunctionType.Silu
    MULT = mybir.AluOpType.mult

    nc = bass.Bass(trn_type="TRN2", name=f"moe_bf16_c{C}")

    # --- DRAM tensors, all host-packed to [128, ...] partition-major bf16 ---
    xe3 = nc.dram_tensor("xe3", [P, KH, C], bf16, kind="ExternalInput")
    wgu = nc.dram_tensor("wgu", [P, 8, 2, KH, 512], bf16, kind="ExternalInput")
    wd3 = nc.dram_tensor("wd3", [P, 4, MI, 512], bf16, kind="ExternalInput")
    x3 = nc.dram_tensor("x3", [P, NCH, KH, NT], bf16, kind="ExternalInput")
    wsg3 = nc.dram_tensor("wsg3", [P, KH, ISH], bf16, kind="ExternalInput")
    wsu3 = nc.dram_tensor("wsu3", [P, KH, ISH], bf16, kind="ExternalInput")
    wsd3 = nc.dram_tensor("wsd3", [P, KSH, H], bf16, kind="ExternalInput")
    ro = nc.dram_tensor("ro", [P, KH, C], bf16, kind="ExternalOutput")
    sp = nc.dram_tensor("sp", [P, NCH, KH, NT], bf16, kind="ExternalOutput")

    runs = _pack_runs(C)

    with tile.TileContext(nc) as tc:
        from contextlib import ExitStack

        with ExitStack() as ctx:
            const = ctx.enter_context(tc.tile_pool(name="const", bufs=1))
            xpool = ctx.enter_context(tc.tile_pool(name="xpool", bufs=2))
            wpool = ctx.enter_context(tc.tile_pool(name="wpool", bufs=8))
            wdpool = ctx.enter_context(tc.tile_pool(name="wdpool", bufs=5))
            hsp = ctx.enter_context(tc.tile_pool(name="hsp", bufs=2))
            hbuf = ctx.enter_context(tc.tile_pool(name="hbuf", bufs=2))
            outp = ctx.enter_context(tc.tile_pool(name="outp", bufs=2))
            psum = ctx.enter_context(tc.tile_pool(name="psum", bufs=8, space="PSUM"))
            hTp = ctx.enter_context(tc.tile_pool(name="hTp", bufs=1))

            # DMA ring split: gate/up weight stream on the SP HWDGE ring
            # (nc.sync, ~32 MB), input loads + down-proj weight stream on the
            # ACT HWDGE ring (nc.scalar, ~26 MB), outputs on SWDGE
            # (nc.gpsimd, ~4.6 MB) — three independent rings so fixed costs
            # overlap and aggregate BW approaches the HBM limit.

            # --- early input loads, in tensor-consumption order ---
            xeT = const.tile([P, KH, C], bf16)
            nc.scalar.dma_start(out=xeT, in_=xe3.ap())
            xts = [None] * NCH

            def load_x(t):
                xts[t] = xpool.tile([P, KH, NT], bf16, tag="xt", name=f"xt{t}")
                nc.scalar.dma_start(out=xts[t], in_=x3.ap()[:, t])

            load_x(0)
            wsg_sb = const.tile([P, KH, ISH], bf16)
            nc.scalar.dma_start(out=wsg_sb, in_=wsg3.ap())
            wsu_sb = const.tile([P, KH, ISH], bf16)
            nc.scalar.dma_start(out=wsu_sb, in_=wsu3.ap())
            load_x(1)
            wsd_sb = const.tile([P, KSH, H], bf16)
            nc.scalar.dma_start(out=wsd_sb, in_=wsd3.ap())

            hT = hTp.tile([P, MI, C], bf16)
            ro_sb = const.tile([P, KH, C], bf16, name="ro_sb")

            # --- PE warm-up: ~30 throwaway matmuls on the routed tokens ---
            # The PE clock boots throttled (K=4/8, 1.2 GHz) and needs ~3.4 us
            # of sustained activity to reach 2.4 GHz. The head of this kernel
            # is DMA-arrival-bound anyway, so spend it warming the clock; the
            # results land in a scratch PSUM slot and are never read. Real
            # accumulations later reclaim the bank via start=True (which
            # clears has_written), so the residue is harmless.
            warm = psum.tile([P, C], f32, tag="ps", name="warm")
            for _ in range(30):
                nc.tensor.matmul(warm, xeT[:, 0, 0:P], xeT[:, 0, :],
                                 start=True, stop=True)

            KHH = KH // 2  # half-group k tiles (1 MB DMA granularity)

            def routed_gu_group(g):
                """Routed expert gate/up for one 512-wide I group -> hT.

                Weights stream in four 1 MB half-tiles (gate k0-7, gate k8-15,
                up k0-7, up k8-15) in consumption order, so the matmuls start
                as soon as the first megabyte lands."""
                halves = []
                for w in range(2):
                    for hh in range(2):
                        wB = wpool.tile([P, KHH, 512], bf16, tag="w",
                                        name=f"w{g}_{w}_{hh}")
                        nc.sync.dma_start(
                            out=wB,
                            in_=wgu.ap()[:, g, w, hh * KHH:(hh + 1) * KHH, :])
                        halves.append(wB)
                gps, ups, mmap = [], [], []
                for ri, q in enumerate(runs):
                    gps.append(psum.tile([P, q, C], f32, tag="ps",
                                         name=f"gps{g}_{ri}"))
                    ups.append(psum.tile([P, q, C], f32, tag="ps",
                                         name=f"ups{g}_{ri}"))
                    for j in range(q):
                        mmap.append((ri, j))
                for w, ps_tiles in ((0, gps), (1, ups)):
                    for k in range(KH):
                        wB = halves[w * 2 + k // KHH]
                        for mi in range(4):
                            ri, j = mmap[mi]
                            # start only on the first write to each PSUM bank:
                            # start=True clears the whole bank's has_written.
                            st = dict(start=(k == 0 and j == 0),
                                      stop=(k == KH - 1))
                            nc.tensor.matmul(ps_tiles[ri][:, j, :],
                                             wB[:, k % KHH, mi * P:(mi + 1) * P],
                                             xeT[:, k, :], **st)
                off = 0
                for ri, q in enumerate(runs):
                    h_sb = hbuf.tile([P, q, C], bf16, tag="hrb",
                                     name=f"hrb{g}_{ri}")
                    nc.scalar.activation(out=h_sb, in_=gps[ri], func=SILU)
                    nc.vector.tensor_tensor(hT[:, g * 4 + off:g * 4 + off + q, :],
                                            h_sb, ups[ri], MULT)
                    off += q

            hss = [None] * NCH

            def shared_gu(t):
                """Shared expert gate/up for one 256-token chunk -> hs[t]."""
                xt = xts[t]
                # pack two [P, NT] fp32 accumulators per PSUM bank
                sg = [psum.tile([P, 2, NT], f32, tag="ps", name=f"sg{t}_{r}")
                      for r in range(2)]
                su = [psum.tile([P, 2, NT], f32, tag="ps", name=f"su{t}_{r}")
                      for r in range(2)]
                for ps_tiles, wB in ((sg, wsg_sb), (su, wsu_sb)):
                    for k in range(KH):
                        for m in range(KSH):
                            st = dict(start=(k == 0 and m % 2 == 0),
                                      stop=(k == KH - 1))
                            nc.tensor.matmul(ps_tiles[m // 2][:, m % 2, :],
                                             wB[:, k, m * P:(m + 1) * P],
                                             xt[:, k, :], **st)
                hs = hsp.tile([P, KSH, NT], bf16, tag="hs", name=f"hs{t}")
                hss[t] = hs
                for r in range(2):
                    htmp = hbuf.tile([P, 2, NT], bf16, tag="hsb",
                                     name=f"htmp{t}_{r}")
                    nc.scalar.activation(out=htmp, in_=sg[r], func=SILU)
                    nc.vector.tensor_tensor(hs[:, 2 * r:2 * r + 2, :], htmp,
                                            su[r], MULT)

            def shared_down(t):
                """Shared expert down-proj for chunk t -> sp."""
                hs = hss[t]
                sp_sb = outp.tile([P, KH, NT], bf16, tag="spsb", name=f"spsb{t}",
                                  bufs=1)
                for m2 in range(KH):
                    ps = psum.tile([P, NT], f32, tag="ps", name=f"sps{t}_{m2}")
                    for k2 in range(KSH):
                        nc.tensor.matmul(ps, wsd_sb[:, k2, m2 * P:(m2 + 1) * P],
                                         hs[:, k2, :],
                                         start=(k2 == 0), stop=(k2 == KSH - 1))
                    nc.vector.tensor_copy(out=sp_sb[:, m2, :], in_=ps)
                nc.gpsimd.dma_start(out=sp.ap()[:, t], in_=sp_sb)

            # wd streams as 16 x 1 MB chunks (8 k2-tiles each) through a
            # 4-deep rolling prefetch: chunks for down-group g2+1 are
            # triggered at the start of group g2, and the first four are
            # hoisted into the gate/up phase (see program order below).
            wd_tiles = {}

            def prefetch_wd(c):
                if c in wd_tiles or c >= 16:
                    return
                wdB = wdpool.tile([P, KHH, 512], bf16, tag="wd",
                                  name=f"wdB{c}")
                # alternate the two HWDGE rings: by the down phase the sync
                # ring has finished the gate/up stream, so both rings fetch
                # wd chunks concurrently.
                eng = nc.scalar if c % 2 == 0 else nc.sync
                g2, q = c // 4, c % 4
                eng.dma_start(
                    out=wdB, in_=wd3.ap()[:, g2, q * KHH:(q + 1) * KHH, :])
                wd_tiles[c] = wdB

            def routed_down_group(g2):
                """Routed expert down-proj for one 512-wide H group -> ro_sb."""
                # queue every remaining wd trigger; each fires as its ring
                # slot frees, so both HWDGE rings stay fed to the end.
                for c in range(4, 16):
                    prefetch_wd(c)
                dps, mmap = [], []
                for ri, q in enumerate(runs):
                    dps.append(psum.tile([P, q, C], f32, tag="ps",
                                         name=f"dps{g2}_{ri}"))
                    for j in range(q):
                        mmap.append((ri, j))
                for k2 in range(MI):
                    wb = wd_tiles[g2 * 4 + k2 // KHH]
                    kk = k2 % KHH
                    for mi in range(4):
                        ri, j = mmap[mi]
                        st = dict(start=(k2 == 0 and j == 0),
                                  stop=(k2 == MI - 1))
                        nc.tensor.matmul(dps[ri][:, j, :],
                                         wb[:, kk, mi * P:(mi + 1) * P],
                                         hT[:, k2, :], **st)
                off = 0
                for ri, q in enumerate(runs):
                    nc.vector.tensor_copy(out=ro_sb[:, g2 * 4 + off:
                                                    g2 * 4 + off + q, :],
                                          in_=dps[ri])
                    off += q
                nc.gpsimd.dma_start(out=ro.ap()[:, g2 * 4:(g2 + 1) * 4, :],
                                    in_=ro_sb[:, g2 * 4:(g2 + 1) * 4, :])

            # Interleave: routed groups are DMA-heavy (4 MB / 7.7 us tensor),
            # shared units are tensor-heavy (resident weights). Spreading the
            # shared work between routed groups keeps both the DMA rings and
            # the PE busy end-to-end; shared_down(3) fills the down-phase
            # DMA-starvation gap.
            routed_gu_group(0)
            shared_gu(0)
            load_x(2)
            routed_gu_group(1)
            shared_down(0)
            routed_gu_group(2)
            shared_gu(1)
            routed_gu_group(3)
            shared_down(1)
            routed_gu_group(4)
            shared_gu(2)
            load_x(3)
            routed_gu_group(5)
            shared_down(2)
            routed_gu_group(6)
            shared_gu(3)
            for c in range(4):
                prefetch_wd(c)
            routed_gu_group(7)
            # down phase: the rolling wd prefetch keeps both rings streaming;
            # shared_down(3) fills the first starvation window.
            routed_down_group(0)
            shared_down(3)
            routed_down_group(1)
            routed_down_group(2)
            routed_down_group(3)

    # Split surplus semaphore waits onto InstEventSemaphore carriers (walrus
    # has a 1-wait limit per instruction).
    import bass_rust
    bass_rust.generate_event_semaphores(nc)
    return nc


def _get_bass(C):
    if C not in _BASS_CACHE:
        _BASS_CACHE[C] = _build_bass(C)
    return _BASS_CACHE[C]


def kernel(**inputs):
    global LAST_RESULT, LAST_NC
    x = np.ascontiguousarray(np.asarray(inputs["x"], dtype=np.float32))
    w_router = np.asarray(inputs["w_router"], dtype=np.float32)
    ws_gate = np.asarray(inputs["ws_gate"], dtype=np.float32)
    ws_up = np.asarray(inputs["ws_up"], dtype=np.float32)
    ws_down = np.asarray(inputs["ws_down"], dtype=np.float32)
    we_gate = np.asarray(inputs["we_gate"], dtype=np.float32)
    we_up = np.asarray(inputs["we_up"], dtype=np.float32)
    we_down = np.asarray(inputs["we_down"], dtype=np.float32)

    # --- top-1 routing on host (tiny) ---
    logits = x @ w_router                      # [T, E]
    top = np.argmax(logits, axis=1)            # [T]
    tv = logits[np.arange(T), top]
    score = (1.0 / (1.0 + np.exp(-tv))).astype(np.float32)
    idxs = [np.nonzero(top == e)[0] for e in range(E)]
    maxn = max(len(i) for i in idxs)
    C = max(P, ((maxn + 15) // 16) * 16)

    nc = _get_bass(C)
    LAST_NC = nc

    # x3[p, t, k, j] = x[t*NT + j, k*128 + p]
    x3 = x.reshape(NCH, NT, KH, P).transpose(3, 0, 2, 1).astype(BF)

    in_maps = []
    for e in range(E):
        idx = idxs[e]
        xe = np.zeros((C, H), np.float32)
        if len(idx):
            xe[:len(idx)] = x[idx] * score[idx, None]
        # xe3[p, k, c] = xe[c, k*128 + p]
        xe3 = xe.reshape(C, KH, P).transpose(2, 1, 0).astype(BF)

        # wgu[p, g, w, k, j] = we_{gate,up}[e][k*128 + p, g*512 + j]
        wgu = np.empty((P, 8, 2, KH, 512), BF)
        wgu[:, :, 0] = we_gate[e].reshape(KH, P, 8, 512).transpose(1, 2, 0, 3)
        wgu[:, :, 1] = we_up[e].reshape(KH, P, 8, 512).transpose(1, 2, 0, 3)
        # wd3[p, g2, k2, j] = we_down[e][k2*128 + p, g2*512 + j]
        wd3 = we_down[e].reshape(MI, P, 4, 512).transpose(1, 2, 0, 3).astype(BF)

        # shared-expert shard for this core
        wsg3 = ws_gate[:, e * ISH:(e + 1) * ISH].reshape(KH, P, ISH) \
            .transpose(1, 0, 2).astype(BF)
        wsu3 = ws_up[:, e * ISH:(e + 1) * ISH].reshape(KH, P, ISH) \
            .transpose(1, 0, 2).astype(BF)
        wsd3 = ws_down[e * ISH:(e + 1) * ISH].reshape(KSH, P, H) \
            .transpose(1, 0, 2).astype(BF)

        in_maps.append({
            "xe3": xe3, "wgu": wgu, "wd3": wd3, "x3": x3,
            "wsg3": wsg3, "wsu3": wsu3, "wsd3": wsd3,
        })

    from concourse.bass_utils import run_bass_kernel_spmd
    res = run_bass_kernel_spmd(nc, in_maps, core_ids=list(range(E)))
    LAST_RESULT = res
    outs = res.results

    # shared partials: sp[p, t, m2, j] -> [token, h], summed over cores
    spsum = np.zeros((P, NCH, KH, NT), np.float32)
    for e in range(E):
        spsum += outs[e]["sp"].astype(np.float32)
    out = np.ascontiguousarray(
        spsum.transpose(1, 3, 2, 0).reshape(T, H))

    # routed: ro[p, m, c] -> [c, h], scatter back by token index
    for e in range(E):
        idx = idxs[e]
        if len(idx):
            roe = outs[e]["ro"].astype(np.float32)
            out[idx] += roe.transpose(2, 1, 0).reshape(C, H)[:len(idx)]
    return out


# revision 24
# speedup vs baseline: 2.1295x; 1.0047x over previous
"""Llama4-style MoE (T=1024, H=2048, I=4096, E=8, top-1) on 8 trn2 NeuronCores.

Sharding: expert-parallel. Core e owns expert e's weights plus a 1/8 I-shard
of the shared expert. Host computes top-1 routing (tiny [1024,8] matmul) and
dispatches each expert's tokens (scaled by the sigmoid router score, padded to
capacity C) to its core. Each core returns its expert's MLP output plus a
partial shared-expert output; host sums the partials and scatters the routed
rows back.

All device-side data is bf16 (weights are streamed once, so HBM traffic is the
roofline: ~63 MB/core ≈ 176 us at 358 GB/s; tensor time at C=144 is ~174 us —
balanced). Host pre-packs every tensor into [128 partitions, ...] layout with
contiguous per-partition lines so each weight DMA is a single 1-4 MB transfer.
Activations live in "transposed" space ([feature, token], feature on
partitions) so no on-chip transposes are needed. PSUM accumulates fp32.

Program order puts the shared expert first: its DMA footprint is small
(~11 MB) but its tensor work is large (~82 us), so the big routed weight
stream (~48 MB) flows underneath it.
"""

import numpy as np
import ml_dtypes

T, H, I, E = 1024, 2048, 4096, 8
P = 128
ISH = I // E          # 512  shared-expert I-shard per core
KH = H // P           # 16
MI = I // P           # 32
KSH = ISH // P        # 4
NT = 256              # shared-expert token chunk
NCH = T // NT         # 4 chunks

BF = ml_dtypes.bfloat16

_BASS_CACHE = {}
LAST_RESULT = None    # BassKernelResults of the most recent run (for test harness)
LAST_NC = None


def _pack_runs(C):
    """Split the 4 m-tiles of a 512-wide group into runs that each fit one
    2 KB PSUM bank ([P, q, C] fp32 with q*C <= 512)."""
    pack = max(1, 512 // C)
    runs = []
    left = 4
    while left:
        q = min(pack, left)
        runs.append(q)
        left -= q
    return runs


def _build_bass(C):
    import concourse.bass as bass
    import concourse.mybir as mybir
    import concourse.tile as tile

    assert C <= 512, f"routed capacity {C} > 512 unsupported"

    f32 = mybir.dt.float32
    bf16 = mybir.dt.bfloat16
    SILU = mybir.ActivationFunctionType.Silu
    MULT = mybir.AluOpType.mult

    nc = bass.Bass(trn_type="TRN2", name=f"moe_bf16_c{C}")

    # --- DRAM tensors, all host-packed to [128, ...] partition-major bf16 ---
    xe3 = nc.dram_tensor("xe3", [P, KH, C], bf16, kind="ExternalInput")
    wgu = nc.dram_tensor("wgu", [P, 8, 2, KH, 512], bf16, kind="ExternalInput")
    wd3 = nc.dram_tensor("wd3", [P, 4, MI, 512], bf16, kind="ExternalInput")
    x3 = nc.dram_tensor("x3", [P, NCH, KH, NT], bf16, kind="ExternalInput")
    wsg3 = nc.dram_tensor("wsg3", [P, KH, ISH], bf16, kind="ExternalInput")
    wsu3 = nc.dram_tensor("wsu3", [P, KH, ISH], bf16, kind="ExternalInput")
    wsd3 = nc.dram_tensor("wsd3", [P, KSH, H], bf16, kind="ExternalInput")
    ro = nc.dram_tensor("ro", [P, KH, C], bf16, kind="ExternalOutput")
    sp = nc.dram_tensor("sp", [P, NCH, KH, NT], bf16, kind="ExternalOutput")

    runs = _pack_runs(C)

    with tile.TileContext(nc) as tc:
        from contextlib import ExitStack

        with ExitStack() as ctx:
            const = ctx.enter_context(tc.tile_pool(name="const", bufs=1))
            xpool = ctx.enter_context(tc.tile_pool(name="xpool", bufs=2))
            wpool = ctx.enter_context(tc.tile_pool(name="wpool", bufs=8))
            wdpool = ctx.enter_context(tc.tile_pool(name="wdpool", bufs=5))
            hsp = ctx.enter_context(tc.tile_pool(name="hsp", bufs=2))
            hbuf = ctx.enter_context(tc.tile_pool(name="hbuf", bufs=2))
            outp = ctx.enter_context(tc.tile_pool(name="outp", bufs=2))
            psum = ctx.enter_context(tc.tile_pool(name="psum", bufs=8, space="PSUM"))
            hTp = ctx.enter_context(tc.tile_pool(name="hTp", bufs=1))

            # DMA ring split: gate/up weight stream on the SP HWDGE ring
            # (nc.sync, ~32 MB), input loads + down-proj weight stream on the
            # ACT HWDGE ring (nc.scalar, ~26 MB), outputs on SWDGE
            # (nc.gpsimd, ~4.6 MB) — three independent rings so fixed costs
            # overlap and aggregate BW approaches the HBM limit.

            # --- early input loads, in tensor-consumption order ---
            xeT = const.tile([P, KH, C], bf16)
            nc.scalar.dma_start(out=xeT, in_=xe3.ap())
            xts = [None] * NCH

            def load_x(t):
                xts[t] = xpool.tile([P, KH, NT], bf16, tag="xt", name=f"xt{t}")
                nc.scalar.dma_start(out=xts[t], in_=x3.ap()[:, t])

            load_x(0)
            wsg_sb = const.tile([P, KH, ISH], bf16)
            nc.scalar.dma_start(out=wsg_sb, in_=wsg3.ap())
            wsu_sb = const.tile([P, KH, ISH], bf16)
            nc.scalar.dma_start(out=wsu_sb, in_=wsu3.ap())
            load_x(1)
            wsd_sb = const.tile([P, KSH, H], bf16)
            nc.scalar.dma_start(out=wsd_sb, in_=wsd3.ap())

            hT = hTp.tile([P, MI, C], bf16)
            ro_sb = const.tile([P, KH, C], bf16, name="ro_sb")

            # --- PE warm-up: ~30 throwaway matmuls on the routed tokens ---
            # The PE clock boots throttled (K=4/8, 1.2 GHz) and needs ~3.4 us
            # of sustained activity to reach 2.4 GHz. The head of this kernel
            # is DMA-arrival-bound anyway, so spend it warming the clock; the
            # results land in a scratch PSUM slot and are never read. Real
            # accumulations later reclaim the bank via start=True (which
            # clears has_written), so the residue is harmless.
            warm = psum.tile([P, C], f32, tag="ps", name="warm")
            for _ in range(30):
                nc.tensor.matmul(warm, xeT[:, 0, 0:P], xeT[:, 0, :],
                                 start=True, stop=True)

            KHH = KH // 2  # half-group k tiles (1 MB DMA granularity)

            def routed_gu_group(g):
                """Routed expert gate/up for one 512-wide I group -> hT.

                Weights stream in four 1 MB half-tiles (gate k0-7, gate k8-15,
                up k0-7, up k8-15) in consumption order, so the matmuls start
                as soon as the first megabyte lands."""
                halves = []
                for w in range(2):
                    for hh in range(2):
                        wB = wpool.tile([P, KHH, 512], bf16, tag="w",
                                        name=f"w{g}_{w}_{hh}")
                        nc.sync.dma_start(
                            out=wB,
                            in_=wgu.ap()[:, g, w, hh * KHH:(hh + 1) * KHH, :])
                        halves.append(wB)
                gps, ups, mmap = [], [], []
                for ri, q in enumerate(runs):
                    gps.append(psum.tile([P, q, C], f32, tag="ps",
                                         name=f"gps{g}_{ri}"))
                    ups.append(psum.tile([P, q, C], f32, tag="ps",
                                         name=f"ups{g}_{ri}"))
                    for j in range(q):
                        mmap.append((ri, j))
                for w, ps_tiles in ((0, gps), (1, ups)):
                    for k in range(KH):
                        wB = halves[w * 2 + k // KHH]
                        for mi in range(4):
                            ri, j = mmap[mi]
                            # start only on the first write to each PSUM bank:
                            # start=True clears the whole bank's has_written.
                            st = dict(start=(k == 0 and j == 0),
                                      stop=(k == KH - 1))
                            nc.tensor.matmul(ps_tiles[ri][:, j, :],
                                             wB[:, k % KHH, mi * P:(mi + 1) * P],
                                             xeT[:, k, :], **st)
                off = 0
                for ri, q in enumerate(runs):
                    h_sb = hbuf.tile([P, q, C], bf16, tag="hrb",
                                     name=f"hrb{g}_{ri}")
                    nc.scalar.activation(out=h_sb, in_=gps[ri], func=SILU)
                    nc.vector.tensor_tensor(hT[:, g * 4 + off:g * 4 + off + q, :],
                                            h_sb, ups[ri], MULT)
                    off += q

            hss = [None] * NCH

            def shared_gu(t):
                """Shared expert gate/up for one 256-token chunk -> hs[t]."""
                xt = xts[t]
                # pack two [P, NT] fp32 accumulators per PSUM bank
                sg = [psum.tile([P, 2, NT], f32, tag="ps", name=f"sg{t}_{r}")
                      for r in range(2)]
                su = [psum.tile([P, 2, NT], f32, tag="ps", name=f"su{t}_{r}")
                      for r in range(2)]
                for ps_tiles, wB in ((sg, wsg_sb), (su, wsu_sb)):
                    for k in range(KH):
                        for m in range(KSH):
                            st = dict(start=(k == 0 and m % 2 == 0),
                                      stop=(k == KH - 1))
                            nc.tensor.matmul(ps_tiles[m // 2][:, m % 2, :],
                                             wB[:, k, m * P:(m + 1) * P],
                                             xt[:, k, :], **st)
                hs = hsp.tile([P, KSH, NT], bf16, tag="hs", name=f"hs{t}")
                hss[t] = hs
                for r in range(2):
                    htmp = hbuf.tile([P, 2, NT], bf16, tag="hsb",
                                     name=f"htmp{t}_{r}")
                    nc.scalar.activation(out=htmp, in_=sg[r], func=SILU)
                    nc.vector.tensor_tensor(hs[:, 2 * r:2 * r + 2, :], htmp,
                                            su[r], MULT)

            def shared_down(t):
                """Shared expert down-proj for chunk t -> sp."""
                hs = hss[t]
                sp_sb = outp.tile([P, KH, NT], bf16, tag="spsb", name=f"spsb{t}",
                                  bufs=1)
                for m2 in range(KH):
                    ps = psum.tile([P, NT], f32, tag="ps", name=f"sps{t}_{m2}")
                    for k2 in range(KSH):
                        nc.tensor.matmul(ps, wsd_sb[:, k2, m2 * P:(m2 + 1) * P],
                                         hs[:, k2, :],
                                         start=(k2 == 0), stop=(k2 == KSH - 1))
                    nc.vector.tensor_copy(out=sp_sb[:, m2, :], in_=ps)
                nc.gpsimd.dma_start(out=sp.ap()[:, t], in_=sp_sb)

            # wd streams as 16 x 1 MB chunks (8 k2-tiles each) through a
            # 4-deep rolling prefetch: chunks for down-group g2+1 are
            # triggered at the start of group g2, and the first four are
            # hoisted into the gate/up phase (see program order below).
            wd_tiles = {}

            def prefetch_wd(c):
                if c in wd_tiles or c >= 16:
                    return
                wdB = wdpool.tile([P, KHH, 512], bf16, tag="wd",
                                  name=f"wdB{c}")
                # alternate the two HWDGE rings: by the down phase the sync
                # ring has finished the gate/up stream, so both rings fetch
                # wd chunks concurrently.
                eng = nc.scalar if c % 2 == 0 else nc.sync
                g2, q = c // 4, c % 4
                eng.dma_start(
                    out=wdB, in_=wd3.ap()[:, g2, q * KHH:(q + 1) * KHH, :])
                wd_tiles[c] = wdB

            def routed_down_group(g2):
                """Routed expert down-proj for one 512-wide H group -> ro_sb."""
                # queue every remaining wd trigger; each fires as its ring
                # slot frees, so both HWDGE rings stay fed to the end.
                for c in range(4, 16):
                    prefetch_wd(c)
                dps, mmap = [], []
                for ri, q in enumerate(runs):
                    dps.append(psum.tile([P, q, C], f32, tag="ps",
                                         name=f"dps{g2}_{ri}"))
                    for j in range(q):
                        mmap.append((ri, j))
                for k2 in range(MI):
                    wb = wd_tiles[g2 * 4 + k2 // KHH]
                    kk = k2 % KHH
                    for mi in range(4):
                        ri, j = mmap[mi]
                        st = dict(start=(k2 == 0 and j == 0),
                                  stop=(k2 == MI - 1))
                        nc.tensor.matmul(dps[ri][:, j, :],
                                         wb[:, kk, mi * P:(mi + 1) * P],
                                         hT[:, k2, :], **st)
                off = 0
                for ri, q in enumerate(runs):
                    nc.vector.tensor_copy(out=ro_sb[:, g2 * 4 + off:
                                                    g2 * 4 + off + q, :],
                                          in_=dps[ri])
                    off += q
                nc.gpsimd.dma_start(out=ro.ap()[:, g2 * 4:(g2 + 1) * 4, :],
                                    in_=ro_sb[:, g2 * 4:(g2 + 1) * 4, :])

            # Interleave: routed groups are DMA-heavy (4 MB / 7.7 us tensor),
            # shared units are tensor-heavy (resident weights). Spreading the
            # shared work between routed groups keeps both the DMA rings and
            # the PE busy end-to-end; shared_down(3) fills the down-phase
            # DMA-starvation gap.
            routed_gu_group(0)
            shared_gu(0)
            load_x(2)
            routed_gu_group(1)
            shared_down(0)
            routed_gu_group(2)
            shared_gu(1)
            routed_gu_group(3)
            shared_down(1)
            routed_gu_group(4)
            shared_gu(2)
            load_x(3)
            routed_gu_group(5)
            shared_down(2)
            routed_gu_group(6)
            shared_gu(3)
            for c in range(4):
                prefetch_wd(c)
            routed_gu_group(7)
            # down phase: the rolling wd prefetch keeps both rings streaming;
            # shared_down(3) fills the first starvation window.
            routed_down_group(0)
            routed_down_group(1)
            routed_down_group(2)
            shared_down(3)
            routed_down_group(3)

    # Split surplus semaphore waits onto InstEventSemaphore carriers (walrus
    # has a 1-wait limit per instruction).
    import bass_rust
    bass_rust.generate_event_semaphores(nc)
    return nc


def _get_bass(C):
    if C not in _BASS_CACHE:
        _BASS_CACHE[C] = _build_bass(C)
    return _BASS_CACHE[C]


def kernel(**inputs):
    global LAST_RESULT, LAST_NC
    x = np.ascontiguousarray(np.asarray(inputs["x"], dtype=np.float32))
    w_router = np.asarray(inputs["w_router"], dtype=np.float32)
    ws_gate = np.asarray(inputs["ws_gate"], dtype=np.float32)
    ws_up = np.asarray(inputs["ws_up"], dtype=np.float32)
    ws_down = np.asarray(inputs["ws_down"], dtype=np.float32)
    we_gate = np.asarray(inputs["we_gate"], dtype=np.float32)
    we_up = np.asarray(inputs["we_up"], dtype=np.float32)
    we_down = np.asarray(inputs["we_down"], dtype=np.float32)

    # --- top-1 routing on host (tiny) ---
    logits = x @ w_router                      # [T, E]
    top = np.argmax(logits, axis=1)            # [T]
    tv = logits[np.arange(T), top]
    score = (1.0 / (1.0 + np.exp(-tv))).astype(np.float32)
    idxs = [np.nonzero(top == e)[0] for e in range(E)]
    maxn = max(len(i) for i in idxs)
    C = max(P, ((maxn + 15) // 16) * 16)

    nc = _get_bass(C)
    LAST_NC = nc

    # x3[p, t, k, j] = x[t*NT + j, k*128 + p]
    x3 = x.reshape(NCH, NT, KH, P).transpose(3, 0, 2, 1).astype(BF)

    in_maps = []
    for e in range(E):
        idx = idxs[e]
        xe = np.zeros((C, H), np.float32)
        if len(idx):
            xe[:len(idx)] = x[idx] * score[idx, None]
        # xe3[p, k, c] = xe[c, k*128 + p]
        xe3 = xe.reshape(C, KH, P).transpose(2, 1, 0).astype(BF)

        # wgu[p, g, w, k, j] = we_{gate,up}[e][k*128 + p, g*512 + j]
        wgu = np.empty((P, 8, 2, KH, 512), BF)
        wgu[:, :, 0] = we_gate[e].reshape(KH, P, 8, 512).transpose(1, 2, 0, 3)
        wgu[:, :, 1] = we_up[e].reshape(KH, P, 8, 512).transpose(1, 2, 0, 3)
        # wd3[p, g2, k2, j] = we_down[e][k2*128 + p, g2*512 + j]
        wd3 = we_down[e].reshape(MI, P, 4, 512).transpose(1, 2, 0, 3).astype(BF)

        # shared-expert shard for this core
        wsg3 = ws_gate[:, e * ISH:(e + 1) * ISH].reshape(KH, P, ISH) \
            .transpose(1, 0, 2).astype(BF)
        wsu3 = ws_up[:, e * ISH:(e + 1) * ISH].reshape(KH, P, ISH) \
            .transpose(1, 0, 2).astype(BF)
        wsd3 = ws_down[e * ISH:(e + 1) * ISH].reshape(KSH, P, H) \
            .transpose(1, 0, 2).astype(BF)

        in_maps.append({
            "xe3": xe3, "wgu": wgu, "wd3": wd3, "x3": x3,
            "wsg3": wsg3, "wsu3": wsu3, "wsd3": wsd3,
        })

    from concourse.bass_utils import run_bass_kernel_spmd
    res = run_bass_kernel_spmd(nc, in_maps, core_ids=list(range(E)))
    LAST_RESULT = res
    outs = res.results

    # shared partials: sp[p, t, m2, j] -> [token, h], summed over cores
    spsum = np.zeros((P, NCH, KH, NT), np.float32)
    for e in range(E):
        spsum += outs[e]["sp"].astype(np.float32)
    out = np.ascontiguousarray(
        spsum.transpose(1, 3, 2, 0).reshape(T, H))

    # routed: ro[p, m, c] -> [c, h], scatter back by token index
    for e in range(E):
        idx = idxs[e]
        if len(idx):
            roe = outs[e]["ro"].astype(np.float32)
            out[idx] += roe.transpose(2, 1, 0).reshape(C, H)[:len(idx)]
    return out
